# revision 1
# baseline (speedup 1.0000x reference)
"""Trainium2 Bass kernel for nn_LocalMessagePassing (2-pass GNN message passing).

8 NeuronCores, SPMD, data-parallel over molecules (4096 atoms/core):
- species routing via masked PSUM accumulation (4 species matmuls + K=1
  outer-product bias matmuls into the same accumulator)
- celu(z) = max(z,0)+exp(min(z,0))-1, the -1 folded into next-layer bias
- per pass: slice MLP -> bf16 neigh table -> AllGather -> dma_gather of edge
  contributions (dest-sorted, per-32-dest-block padded) -> segment-sum via
  matmul (mergedT += gathered^T @ (onehot*edge_w)) accumulated in PSUM
- final routed linear (M=1 matmuls) + per-molecule charge correction on chip
"""
import sys
sys.path.insert(0, "/opt/trn_rl_repo")
import math
import numpy as np
import ml_dtypes

import concourse.bacc as bacc
import concourse.mybir as mybir
import concourse.tile as tile
from concourse.alu_op_type import AluOpType

BF16 = mybir.dt.bfloat16
FP32 = mybir.dt.float32
I16 = mybir.dt.int16
AF = mybir.ActivationFunctionType
F32R = mybir.dt.float32r

N_CORES = 8
S = 4
CUTOFF = 5.2
D_BLK = 32
GSUB = 7            # 128-idx chunks per dma_gather (57 ring entries)


def split_multi_waits(nc):
    """This walrus build allows one sync-wait per instruction; hoist extras
    onto same-engine NOPs placed immediately before."""
    cnt = 0
    for fn in nc.m.functions:
        for bb in fn.blocks:
            out = []
            changed = False
            for ins in bb.instructions:
                si = ins.sync_info
                if si is not None and len(si.on_wait) > 1:
                    waits = list(si.on_wait)
                    for w in waits[:-1]:
                        cnt += 1
                        out.append(mybir.InstNoOp(
                            name=f"wsplit-{cnt}", engine=ins.engine,
                            bass_nofuse=True,
                            sync_info=mybir.SyncInfo(on_wait=[w], on_update=[]),
                        ))
                    ins.sync_info = mybir.SyncInfo(
                        on_wait=[waits[-1]], on_update=list(si.on_update))
                    changed = True
                out.append(ins)
            if changed:
                bb.instructions = out
    return cnt


# ---------------------------------------------------------------- program
def build_program(apc, f_in, h_dim, mo, no, c_blk):
    nblocks = apc // D_BLK
    n_chunks = nblocks * c_blk
    ngi = (n_chunks + GSUB - 1) // GSUB
    padn_g = ngi * GSUB * 128
    n_tab = N_CORES * apc
    CT = 512
    ncts = apc // CT
    hck = h_dim // 128

    nc = bacc.Bacc("TRN2", target_bir_lowering=False, debug=False,
                   num_devices=N_CORES)

    def din(name, shape, dt):
        return nc.dram_tensor(name, shape, dt, kind="ExternalInput").ap()

    xT_in = din("xT", [f_in, apc], FP32)
    spid_in = din("spid", [128, apc], BF16)
    w_p0w1 = din("w_p0w1", [S, f_in, h_dim], FP32)
    w_p0w2 = din("w_p0w2", [S, 2, 128, mo], FP32)
    w_n0 = din("w_n0", [S, mo, no], FP32)
    w_p1w1 = din("w_p1w1", [S, 2, 128, h_dim], FP32)
    w_p1w2 = din("w_p1w2", [S, 2, 128, mo], FP32)
    w_n1 = din("w_n1", [S, mo, no], FP32)
    w_f = din("w_f", [S, 2, 128, 1], FP32)
    b_p0b1 = din("b_p0b1", [S, 1, h_dim], FP32)
    b_p0b2 = din("b_p0b2", [S, 1, mo], FP32)
    b_n0 = din("b_n0", [S, 1, no], FP32)
    b_p1b1 = din("b_p1b1", [S, 1, h_dim], FP32)
    b_p1b2 = din("b_p1b2", [S, 1, mo], FP32)
    b_n1 = din("b_n1", [S, 1, no], FP32)
    b_f = din("b_f", [S, 1, 1], FP32)
    eidx = din("eidx", [128, padn_g // 16], I16)
    destid = din("destid", [128, n_chunks], BF16)
    edist = din("edist", [128, n_chunks], FP32)
    iota32 = din("iota32", [128, D_BLK], BF16)
    escal = din("escal", [128, 4], FP32)
    tc_in = din("tc_in", [1, apc // 128], FP32)
    idin = din("ident", [128, 128], FP32)

    prech_out = nc.dram_tensor("prech", [1, apc], FP32, kind="ExternalOutput").ap()
    charg_out = nc.dram_tensor("charg", [1, apc], FP32, kind="ExternalOutput").ap()

    ntab_loc = [nc.dram_tensor(f"ntl{p}", [apc, no], FP32).ap() for p in range(2)]
    ntab = [nc.dram_tensor(f"ntab{p}", [n_tab, no], FP32, addr_space="Shared").ap()
            for p in range(2)]

    with tile.TileContext(nc) as tc:
        import contextlib
        with contextlib.ExitStack() as ctx:
            persist = ctx.enter_context(tc.tile_pool(name="persist", bufs=1))
            mlp = ctx.enter_context(tc.tile_pool(name="mlp", bufs=2))
            xmp = ctx.enter_context(tc.tile_pool(name="xmp", bufs=1))
            psA = ctx.enter_context(tc.tile_pool(name="psA", bufs=2, space="PSUM"))
            psT = ctx.enter_context(tc.tile_pool(name="psT", bufs=2, space="PSUM"))
            gat = ctx.enter_context(tc.tile_pool(name="gat", bufs=3))
            mkp = ctx.enter_context(tc.tile_pool(name="mkp", bufs=4))

            def loadp(ap_in, shape, dt, name):
                t = persist.tile(shape, dt, tag=name)
                nc.sync.dma_start(t[:], ap_in)
                return t

            spid_t = loadp(spid_in[:, :], [128, apc], BF16, "spid")
            iota_t = loadp(iota32[:, :], [128, D_BLK], BF16, "iota")
            escal_t = loadp(escal[:, :], [128, 4], FP32, "escal")
            destid_t = loadp(destid[:, :], [128, n_chunks], BF16, "destid")
            edist_t = loadp(edist[:, :], [128, n_chunks], FP32, "edist")
            tc_t = loadp(tc_in[:, :], [1, apc // 128], FP32, "tc")
            ident_t = loadp(idin[:, :], [128, 128], FP32, "ident")

            # simpler: explicit loads
            def wload3(ap_src, k, s_count, m, name):
                # src AP pre-rearranged to [k, s, m]
                t = persist.tile([k, s_count * m], ap_src.dtype, tag=name)
                nc.sync.dma_start(
                    t[:].rearrange("k (s m) -> k s m", s=s_count), ap_src)
                return t

            def wload4(ap_src, k, s_count, c, m, name):
                # src AP pre-rearranged to [k, s, c, m]
                t = persist.tile([k, s_count * c * m], ap_src.dtype, tag=name)
                nc.sync.dma_start(
                    t[:].rearrange("k (s c m) -> k s c m", s=s_count, c=c), ap_src)
                return t

            w1t = [wload3(w_p0w1[:, :, :].rearrange("s k m -> k s m"),
                          f_in, S, h_dim, "w1t0"),
                   wload4(w_p1w1[:, :, :, :].rearrange("s c k m -> k s c m"),
                          128, S, 2, h_dim, "w1t1")]
            w2t = [wload4(w_p0w2[:, :, :, :].rearrange("s c k m -> k s c m"),
                          128, S, 2, mo, "w2t0"),
                   wload4(w_p1w2[:, :, :, :].rearrange("s c k m -> k s c m"),
                          128, S, 2, mo, "w2t1")]
            nwt = [wload3(w_n0[:, :, :].rearrange("s k m -> k s m"),
                          mo, S, no, "nwt0"),
                   wload3(w_n1[:, :, :].rearrange("s k m -> k s m"),
                          mo, S, no, "nwt1")]
            fwt = wload4(w_f[:, :, :, :].rearrange("s c k m -> k s c m"),
                         128, S, 2, 1, "fwt")
            b1t = [wload3(b_p0b1[:, :, :].rearrange("s k m -> k s m"), 1, S, h_dim, "b1t0"),
                   wload3(b_p1b1[:, :, :].rearrange("s k m -> k s m"), 1, S, h_dim, "b1t1")]
            b2t = [wload3(b_p0b2[:, :, :].rearrange("s k m -> k s m"), 1, S, mo, "b2t0"),
                   wload3(b_p1b2[:, :, :].rearrange("s k m -> k s m"), 1, S, mo, "b2t1")]
            nbt = [wload3(b_n0[:, :, :].rearrange("s k m -> k s m"), 1, S, no, "nbt0"),
                   wload3(b_n1[:, :, :].rearrange("s k m -> k s m"), 1, S, no, "nbt1")]
            fbt = wload3(b_f[:, :, :].rearrange("s k m -> k s m"), 1, S, 1, "fbt")

            # edge weights
            wtile = persist.tile([128, n_chunks], BF16, tag="wtile")
            dec = mlp.tile([128, n_chunks], FP32, tag="dec", bufs=1)
            nc.scalar.activation(dec[:], edist_t[:], AF.Exp,
                                 bias=escal_t[:, 1:2], scale=escal_t[:, 0:1])
            cut = mlp.tile([128, n_chunks], FP32, tag="cut", bufs=1)
            nc.scalar.activation(cut[:], edist_t[:], AF.Sin,
                                 bias=escal_t[:, 2:3], scale=math.pi / CUTOFF)
            nc.vector.tensor_scalar_add(cut[:], cut[:], 1.0)
            nc.vector.tensor_tensor(wtile[:], dec[:], cut[:], AluOpType.mult)

            internT = [persist.tile([mo, apc], FP32, tag=f"internT{p}",
                                    name=f"internT{p}")
                       for p in range(2)]
            mergedT = persist.tile([no, apc], FP32, tag="mergedT")

            mskp = ctx.enter_context(tc.tile_pool(name="mskp", bufs=1))
            idxp = ctx.enter_context(tc.tile_pool(name="idxp", bufs=8))
            _cur_msk = {}

            def build_masks(ct):
                for s in range(S):
                    mt = mskp.tile([128, CT], FP32, tag=f"msk{s}", name=f"msk{s}")
                    nc.vector.tensor_scalar(
                        mt[:], spid_t[:, ct * CT:ct * CT + CT], float(s), None,
                        AluOpType.is_equal)
                    _cur_msk[s] = mt

            def msl(s, ct):
                return _cur_msk[s][:]

            def msl0(s, ct):
                return _cur_msk[s][0:1, :]

            def mlp_phase(p):
                kck = 1 if p == 0 else 2
                for ct in range(ncts):
                    asl = slice(ct * CT, ct * CT + CT)
                    build_masks(ct)
                    if p == 0:
                        xseg = mlp.tile([128, CT], FP32, tag="xseg")
                        nc.sync.dma_start(xseg[:], xT_in[:, asl])
                    # masked inputs for this column tile
                    xms = {}
                    for s in range(S):
                        for kc in range(kck):
                            xm = xmp.tile([128, CT], FP32, tag=f"xm{s}_{kc}")
                            src = (xseg if p == 0
                                   else (internT[0] if kc == 0 else mergedT))
                            nc.vector.tensor_tensor(
                                xm[:], (src[:] if p == 0 else src[:, asl]),
                                msl(s, ct), AluOpType.mult)
                            xms[s, kc] = xm
                    hts = []
                    for hc in range(hck):
                        z1 = psA.tile([128, CT], FP32, tag="zz")
                        for s in range(S):
                            nc.tensor.matmul(
                                z1[:],
                                b1t[p][0:1, s * h_dim + hc * 128:
                                       s * h_dim + hc * 128 + 128],
                                msl0(s, ct), start=(s == 0), stop=False)
                        for s in range(S):
                            for kc in range(kck):
                                if p == 0:
                                    lhsT = w1t[0][:, s * h_dim + hc * 128:
                                                  s * h_dim + hc * 128 + 128]
                                else:
                                    base = s * 2 * h_dim + kc * h_dim + hc * 128
                                    lhsT = w1t[1][:, base:base + 128]
                                nc.tensor.matmul(
                                    z1[:], lhsT,
                                    xms[s, kc][:],
                                    start=False,
                                    stop=(s == S - 1 and kc == kck - 1))
                        mn = mlp.tile([128, CT], FP32, tag="mn")
                        nc.vector.tensor_scalar_min(mn[:], z1[:], 0.0)
                        ex = mlp.tile([128, CT], FP32, tag="ex")
                        nc.scalar.activation(ex[:], mn[:], AF.Exp)
                        ht = mlp.tile([128, CT], FP32, tag="ht")
                        nc.vector.tensor_scalar_max(ht[:], z1[:], 0.0)
                        nc.vector.tensor_tensor(ht[:], ht[:], ex[:], AluOpType.add)
                        hts.append(ht)
                    z2 = psA.tile([128, CT], FP32, tag="zz")
                    for s in range(S):
                        nc.tensor.matmul(
                            z2[:], b2t[p][0:1, s * mo:s * mo + 128],
                            msl0(s, ct), start=(s == 0), stop=False)
                    for s in range(S):
                        for hc in range(hck):
                            hm = mlp.tile([128, CT], FP32, tag="hm")
                            nc.vector.tensor_tensor(
                                hm[:], hts[hc][:], msl(s, ct), AluOpType.mult)
                            base = s * 2 * mo + hc * mo
                            nc.tensor.matmul(
                                z2[:], w2t[p][:, base:base + 128],
                                hm[:],
                                start=False, stop=(s == S - 1 and hc == hck - 1))
                    nc.vector.tensor_copy(internT[p][:, asl], z2[:])
                    zn = psA.tile([128, CT], FP32, tag="zz")
                    for s in range(S):
                        nc.tensor.matmul(
                            zn[:], nbt[p][0:1, s * no:s * no + 128],
                            msl0(s, ct), start=(s == 0), stop=False)
                    for s in range(S):
                        im = mlp.tile([128, CT], FP32, tag="im")
                        nc.vector.tensor_tensor(
                            im[:], internT[p][:, asl], msl(s, ct), AluOpType.mult)
                        nc.tensor.matmul(
                            zn[:], nwt[p][:, s * no:s * no + 128],
                            im[:],
                            start=False, stop=(s == S - 1))
                    nT = mlp.tile([128, CT], FP32, tag="nT")
                    nc.vector.tensor_copy(nT[:], zn[:])
                    for q in range(CT // 128):
                        pt = psT.tile([128, 128], FP32, tag="pt")
                        nc.tensor.transpose(pt[:], nT[:, q * 128:q * 128 + 128],
                                            ident_t[:])
                        rowt = mlp.tile([128, 128], FP32, tag="rowt")
                        nc.vector.tensor_copy(rowt[:], pt[:])
                        r0 = ct * CT + q * 128
                        nc.sync.dma_start(ntab_loc[p][r0:r0 + 128, :], rowt[:])

            def edge_phase(p):
                pscols = 512
                bpp = pscols // D_BLK       # blocks per psum tile (16)
                psm = None
                for gi in range(ngi):
                    it = idxp.tile([128, GSUB * 8], I16, tag="it")
                    nc.sync.dma_start(it[:], eidx[:, gi * GSUB * 8:(gi + 1) * GSUB * 8])
                    gt = gat.tile([128, GSUB, no], FP32, tag="gt")
                    nc.gpsimd.dma_gather(
                        gt[:], ntab[p][:, :], it[:],
                        num_idxs=GSUB * 128, num_idxs_reg=GSUB * 128,
                        elem_size=no)
                    c0 = gi * GSUB
                    nsub = min(GSUB, n_chunks - c0)
                    if nsub <= 0:
                        continue
                    csl = slice(c0, c0 + nsub)
                    mk = mkp.tile([128, GSUB * D_BLK], BF16, tag="mk")
                    nc.vector.tensor_tensor(
                        mk[:, :nsub * D_BLK].rearrange("p (c d) -> p c d", d=D_BLK),
                        destid_t[:, csl].broadcast_to([128, nsub, D_BLK]),
                        iota_t[:].rearrange("p (x d) -> p x d", x=1)
                              .broadcast_to([128, nsub, D_BLK]),
                        AluOpType.is_equal)
                    wm = mkp.tile([128, GSUB * D_BLK], FP32, tag="wm")
                    nc.vector.tensor_tensor(
                        wm[:, :nsub * D_BLK].rearrange("p (c d) -> p c d", d=D_BLK),
                        mk[:, :nsub * D_BLK].rearrange("p (c d) -> p c d", d=D_BLK),
                        wtile[:, csl].broadcast_to([128, nsub, D_BLK]),
                        AluOpType.mult)
                    for sub in range(nsub):
                        g = c0 + sub
                        blk = g // c_blk
                        cin = g % c_blk
                        if blk % bpp == 0 and cin == 0:
                            psm = psT.tile([128, pscols], FP32, tag="psm")
                        col0 = (blk % bpp) * D_BLK
                        nc.tensor.matmul(
                            psm[:, col0:col0 + D_BLK],
                            gt[:, sub, :],
                            wm[:, sub * D_BLK:sub * D_BLK + D_BLK],
                            start=(cin == 0), stop=(cin == c_blk - 1))
                        if cin == c_blk - 1 and blk % bpp == bpp - 1:
                            grp = blk // bpp
                            nc.vector.tensor_copy(
                                mergedT[:, grp * pscols:(grp + 1) * pscols],
                                psm[:])

            def final_phase():
                prech = persist.tile([1, apc], FP32, tag="prech")
                for ct in range(ncts):
                    build_masks(ct)
                    zf = psT.tile([1, CT], FP32, tag="psm")
                    for s in range(S):
                        nc.tensor.matmul(
                            zf[:], fbt[0:1, s:s + 1], msl0(s, ct),
                            start=(s == 0), stop=False)
                    for s in range(S):
                        for kc in range(2):
                            src = internT[1] if kc == 0 else mergedT
                            xm = mlp.tile([128, CT], FP32, tag="xmf")
                            nc.vector.tensor_tensor(
                                xm[:], src[:, ct * CT:ct * CT + CT],
                                msl(s, ct), AluOpType.mult)
                            nc.tensor.matmul(
                                zf[:], fwt[:, s * 2 + kc:s * 2 + kc + 1],
                                xm[:],
                                start=False,
                                stop=(s == S - 1 and kc == 1))
                    nc.vector.tensor_copy(prech[0:1, ct * CT:ct * CT + CT], zf[:])
                nc.sync.dma_start(prech_out[:, :], prech[:])
                nmol = apc // 128
                tpre = mlp.tile([1, nmol], FP32, tag="tpre")
                nc.vector.tensor_reduce(
                    tpre[0:1, :],
                    prech[0:1, :].rearrange("p (m a) -> p m a", a=128),
                    mybir.AxisListType.X, AluOpType.add)
                corr = mlp.tile([1, nmol], FP32, tag="corr")
                nc.vector.tensor_tensor(corr[:], tc_t[:], tpre[:],
                                        AluOpType.subtract)
                nc.vector.tensor_scalar_mul(corr[:], corr[:], 1.0 / 128.0)
                nc.vector.tensor_tensor(
                    prech[0:1, :].rearrange("p (m a) -> p m a", a=128),
                    prech[0:1, :].rearrange("p (m a) -> p m a", a=128),
                    corr[0:1, :].broadcast_to([1, nmol, 128]),
                    AluOpType.add)
                nc.sync.dma_start(charg_out[:, :], prech[:])

            for p in range(2):
                mlp_phase(p)
                nc.gpsimd.collective_compute(
                    "AllGather", AluOpType.bypass,
                    replica_groups=[list(range(N_CORES))],
                    ins=[ntab_loc[p]], outs=[ntab[p]])
                edge_phase(p)
            final_phase()

    nc.compile()
    split_multi_waits(nc)
    return nc


# ---------------------------------------------------------------- host prep
def _wrap_idx(flat_idx):
    n = len(flat_idx)
    a = np.zeros((16, (n + 15) // 16), np.int16)
    a[np.arange(n) % 16, np.arange(n) // 16] = flat_idx
    return np.tile(a, (8, 1))


def prepare_inputs(species, in_features, atom_index12, distances, total_charges,
                   p0_w1, p0_b1, p0_w2, p0_b2, n0_w, n0_b,
                   p1_w1, p1_b1, p1_w2, p1_b2, n1_w, n1_b,
                   f_w, f_b, prefactor, factor):
    B, A = np.asarray(species).shape
    N = B * A
    F_IN = np.asarray(in_features).shape[-1]
    H = np.asarray(p0_w1).shape[-1]
    MO = np.asarray(p0_w2).shape[-1]
    NO = np.asarray(n0_w).shape[-1]
    APC = N // N_CORES
    sp = np.asarray(species).reshape(-1).astype(np.int64)
    feats = np.asarray(in_features, np.float32).reshape(N, F_IN)

    # atoms keep their natural order (core c owns [c*APC, (c+1)*APC));
    # species routing is fully mask-based so no sorting is needed, and
    # molecule boundaries (128-atom groups) stay intact for the charge
    # correction.
    perm = np.arange(N)
    inv = perm
    sp_sorted = sp

    i0 = inv[np.asarray(atom_index12[0], np.int64)]
    i1 = inv[np.asarray(atom_index12[1], np.int64)]
    dd = np.asarray(distances, np.float32)
    dest = np.concatenate([i0, i1])
    src = np.concatenate([i1, i0])
    ddist = np.concatenate([dd, dd])

    nblocks = APC // D_BLK
    dcore = dest // APC
    dloc = dest - dcore * APC
    dblk = dloc // D_BLK

    counts = np.bincount(dcore * nblocks + dblk, minlength=N_CORES * nblocks)
    c_blk = int(np.ceil(counts.max() / 128.0))
    n_chunks = nblocks * c_blk
    ngi = (n_chunks + GSUB - 1) // GSUB
    padn_g = ngi * GSUB * 128
    slots = n_chunks * 128

    key = dcore * nblocks + dblk
    order = np.argsort(key, kind="stable")
    bounds = np.searchsorted(key[order], np.arange(N_CORES * nblocks + 1))

    eidx_np = np.zeros((N_CORES, 128, padn_g // 16), np.int16)
    destid_np = np.zeros((N_CORES, 128, n_chunks), ml_dtypes.bfloat16)
    edist_np = np.zeros((N_CORES, 128, n_chunks), np.float32)
    j = np.arange(slots)
    for c in range(N_CORES):
        idx_flat = np.zeros(slots, np.int64)
        did_flat = np.full(slots, float(D_BLK), np.float32)   # pad -> no match
        dst_flat = np.zeros(slots, np.float32)
        for b in range(nblocks):
            g0, g1 = bounds[c * nblocks + b], bounds[c * nblocks + b + 1]
            cnt = g1 - g0
            s0 = b * c_blk * 128
            sel = order[g0:g1]
            idx_flat[s0:s0 + cnt] = src[sel]
            did_flat[s0:s0 + cnt] = (dloc[sel] % D_BLK).astype(np.float32)
            dst_flat[s0:s0 + cnt] = ddist[sel]
        eidx_np[c] = _wrap_idx(np.concatenate(
            [idx_flat, np.zeros(padn_g - slots, np.int64)]).astype(np.int16))
        destid_np[c, j % 128, j // 128] = did_flat.astype(ml_dtypes.bfloat16)
        edist_np[c, j % 128, j // 128] = dst_flat

    def bf(x):
        return np.ascontiguousarray(np.asarray(x, np.float32)).astype(
            ml_dtypes.bfloat16)

    def f32(x):
        return np.ascontiguousarray(np.asarray(x, np.float32))

    pf = float(np.asarray(prefactor)); fc = float(np.asarray(factor))
    escal_np = np.zeros((128, 4), np.float32)
    escal_np[:, 2] = math.pi / 2.0
    escal_np[:, 0] = -fc * fc
    escal_np[:, 1] = math.log(max(0.5 * pf * pf, 1e-30))

    p0b2_adj = np.asarray(p0_b2, np.float64) - np.asarray(p0_w2, np.float64).sum(1)
    p1b2_adj = np.asarray(p1_b2, np.float64) - np.asarray(p1_w2, np.float64).sum(1)

    def kchunk(w):  # [S, 2k, m] -> [S, 2, 128, m]
        w = np.asarray(w, np.float32)
        return w.reshape(w.shape[0], 2, 128, w.shape[-1])

    common = {
        "w_p0w1": f32(p0_w1), "w_p0w2": f32(kchunk(p0_w2)), "w_n0": f32(n0_w),
        "w_p1w1": f32(kchunk(p1_w1)), "w_p1w2": f32(kchunk(p1_w2)),
        "w_n1": f32(n1_w), "w_f": f32(kchunk(f_w)),
        "b_p0b1": f32(np.asarray(p0_b1))[:, None, :],
        "b_p0b2": f32(p0b2_adj)[:, None, :],
        "b_n0": f32(np.asarray(n0_b))[:, None, :],
        "b_p1b1": f32(np.asarray(p1_b1))[:, None, :],
        "b_p1b2": f32(p1b2_adj)[:, None, :],
        "b_n1": f32(np.asarray(n1_b))[:, None, :],
        "b_f": f32(np.asarray(f_b))[:, None, :],
        "iota32": np.tile(np.arange(D_BLK, dtype=np.float32).astype(
            ml_dtypes.bfloat16)[None, :], (128, 1)),
        "escal": escal_np,
        "ident": np.eye(128, dtype=np.float32),
    }

    tc_np = np.asarray(total_charges, np.float32)
    in_maps = []
    for c in range(N_CORES):
        asl = slice(c * APC, (c + 1) * APC)
        spc = sp_sorted[asl]
        xT = np.ascontiguousarray(feats[perm[asl]].T)
        spid_c = np.tile(spc.astype(np.float32)[None, :], (128, 1)).astype(
            ml_dtypes.bfloat16)
        in_maps.append({
            "xT": xT, "spid": spid_c,
            "eidx": eidx_np[c], "destid": destid_np[c], "edist": edist_np[c],
            "tc_in": tc_np[c * (B // N_CORES):(c + 1) * (B // N_CORES)][None, :],
            **common,
        })
    meta = dict(perm=perm, B=B, A=A, APC=APC, c_blk=c_blk,
                F_IN=F_IN, H=H, MO=MO, NO=NO)
    return in_maps, meta


# ---------------------------------------------------------------- runner
class SpmdRunner:
    def __init__(self, nc, n_cores=N_CORES):
        import jax
        from concourse import bass2jax
        from concourse.bass2jax import _bass_exec_p, install_neuronx_cc_hook
        from jax.sharding import Mesh, PartitionSpec
        from jax.experimental.shard_map import shard_map
        install_neuronx_cc_hook()
        self.jax = jax
        self.nc = nc
        self.n_cores = n_cores
        in_names, out_names, out_avals, zero_outs = [], [], [], []
        partition_name = (nc.partition_id_tensor.name
                          if nc.partition_id_tensor else None)
        for alloc in nc.m.functions[0].allocations:
            if not isinstance(alloc, mybir.MemoryLocationSet):
                continue
            name = alloc.memorylocations[0].name
            if alloc.kind == "ExternalInput":
                if name != partition_name:
                    in_names.append(name)
            elif alloc.kind == "ExternalOutput":
                shape = tuple(alloc.tensor_shape)
                dtype = mybir.dt.np(alloc.dtype)
                out_names.append(name)
                out_avals.append(jax.core.ShapedArray(shape, dtype))
                zero_outs.append(np.zeros(shape, dtype))
        n_params = len(in_names)
        all_in = in_names + out_names
        if partition_name is not None:
            all_in.append(partition_name)

        def _body(*args):
            operands = list(args)
            if partition_name is not None:
                operands.append(bass2jax.partition_id_tensor())
            outs = _bass_exec_p.bind(
                *operands, out_avals=tuple(out_avals), in_names=tuple(all_in),
                out_names=tuple(out_names), lowering_input_output_aliases=(),
                sim_require_finite=True, sim_require_nnan=True, nc=nc)
            return tuple(outs)

        devices = jax.devices()[:n_cores]
        mesh = Mesh(np.asarray(devices), ("core",))
        in_specs = (PartitionSpec("core"),) * (n_params + len(out_names))
        out_specs = (PartitionSpec("core"),) * len(out_names)
        self._fn = jax.jit(
            shard_map(_body, mesh=mesh, in_specs=in_specs,
                      out_specs=out_specs, check_rep=False),
            keep_unused=True)
        self.in_names, self.out_names = in_names, out_names
        self.out_avals, self.zero_outs = out_avals, zero_outs
        self.n_params = n_params

    def prepare(self, in_maps):
        per_core = [[np.asarray(m[n]) for n in self.in_names] for m in in_maps]
        concat_in = [
            np.concatenate([per_core[c][i] for c in range(self.n_cores)], axis=0)
            for i in range(self.n_params)]
        concat_zeros = [
            np.zeros((self.n_cores * z.shape[0], *z.shape[1:]), z.dtype)
            for z in self.zero_outs]
        args = [self.jax.device_put(a) for a in concat_in + concat_zeros]
        for a in args:
            a.block_until_ready()
        self._args = args

    def run(self):
        outs = self._fn(*self._args)
        self.jax.block_until_ready(outs)
        return outs

    def results(self, outs):
        return [
            {name: np.asarray(outs[i]).reshape(
                self.n_cores, *self.out_avals[i].shape)[c]
             for i, name in enumerate(self.out_names)}
            for c in range(self.n_cores)]


_CACHE = {}


def _get_runner(apc, f_in, h_dim, mo, no, c_blk):
    key = (apc, f_in, h_dim, mo, no, c_blk)
    if key not in _CACHE:
        nc = build_program(apc, f_in, h_dim, mo, no, c_blk)
        _CACHE[key] = SpmdRunner(nc, N_CORES)
    return _CACHE[key]


def kernel(**inputs):
    species = inputs["species"]
    in_maps, meta = prepare_inputs(**inputs)
    r = _get_runner(meta["APC"], meta["F_IN"], meta["H"], meta["MO"],
                    meta["NO"], meta["c_blk"])
    r.prepare(in_maps)
    outs = r.run()
    res = r.results(outs)
    N = meta["B"] * meta["A"]
    prech = np.empty(N, np.float32)
    charg = np.empty(N, np.float32)
    for c in range(N_CORES):
        asl = slice(c * meta["APC"], (c + 1) * meta["APC"])
        prech[meta["perm"][asl]] = res[c]["prech"][0]
        charg[meta["perm"][asl]] = res[c]["charg"][0]
    B, A = meta["B"], meta["A"]
    return species, charg.reshape(B, A), prech.reshape(B, A)



# revision 3
# speedup vs baseline: 2.0920x; 2.0920x over previous
"""Trainium2 Bass kernel for nn_LocalMessagePassing (2-pass GNN message passing).

8 NeuronCores, SPMD, data-parallel over molecules (4096 atoms/core):
- species routing via masked PSUM accumulation (4 species matmuls + K=1
  outer-product bias matmuls into the same accumulator)
- celu(z) = max(z,0)+exp(min(z,0))-1, the -1 folded into next-layer bias
- per pass: slice MLP -> bf16 neigh table -> AllGather -> dma_gather of edge
  contributions (dest-sorted, per-32-dest-block padded) -> segment-sum via
  matmul (mergedT += gathered^T @ (onehot*edge_w)) accumulated in PSUM
- final routed linear (M=1 matmuls) + per-molecule charge correction on chip
"""
import sys
sys.path.insert(0, "/opt/trn_rl_repo")
import math
import numpy as np
import ml_dtypes

import concourse.bacc as bacc
import concourse.mybir as mybir
import concourse.tile as tile
from concourse.alu_op_type import AluOpType

BF16 = mybir.dt.bfloat16
FP32 = mybir.dt.float32
I16 = mybir.dt.int16
AF = mybir.ActivationFunctionType
F32R = mybir.dt.float32r

N_CORES = 8
S = 4
CUTOFF = 5.2
D_BLK = 32
GSUB = 7            # 128-idx chunks per dma_gather (57 ring entries)


def split_multi_waits(nc):
    """This walrus build allows one sync-wait per instruction; hoist extras
    onto same-engine NOPs placed immediately before."""
    cnt = 0
    for fn in nc.m.functions:
        for bb in fn.blocks:
            out = []
            changed = False
            for ins in bb.instructions:
                si = ins.sync_info
                if si is not None and len(si.on_wait) > 1:
                    waits = list(si.on_wait)
                    for w in waits[:-1]:
                        cnt += 1
                        out.append(mybir.InstNoOp(
                            name=f"wsplit-{cnt}", engine=ins.engine,
                            bass_nofuse=True,
                            sync_info=mybir.SyncInfo(on_wait=[w], on_update=[]),
                        ))
                    ins.sync_info = mybir.SyncInfo(
                        on_wait=[waits[-1]], on_update=list(si.on_update))
                    changed = True
                out.append(ins)
            if changed:
                bb.instructions = out
    return cnt


# ---------------------------------------------------------------- program
def build_program(apc, f_in, h_dim, mo, no, c_blk):
    nblocks = apc // D_BLK
    n_chunks = nblocks * c_blk
    ngi = (n_chunks + GSUB - 1) // GSUB
    padn_g = ngi * GSUB * 128
    n_tab = N_CORES * apc
    CT = 512
    ncts = apc // CT
    hck = h_dim // 128

    nc = bacc.Bacc("TRN2", target_bir_lowering=False, debug=False,
                   num_devices=N_CORES)

    def din(name, shape, dt):
        return nc.dram_tensor(name, shape, dt, kind="ExternalInput").ap()

    xT_in = din("xT", [f_in, apc], FP32)
    spid_in = din("spid", [128, apc], BF16)
    w_p0w1 = din("w_p0w1", [S, f_in, h_dim], FP32)
    w_p0w2 = din("w_p0w2", [S, 2, 128, mo], FP32)
    w_n0 = din("w_n0", [S, mo, no], FP32)
    w_p1w1 = din("w_p1w1", [S, 2, 128, h_dim], FP32)
    w_p1w2 = din("w_p1w2", [S, 2, 128, mo], FP32)
    w_n1 = din("w_n1", [S, mo, no], FP32)
    w_f = din("w_f", [S, 2, 128, 1], FP32)
    b_p0b1 = din("b_p0b1", [S, 1, h_dim], FP32)
    b_p0b2 = din("b_p0b2", [S, 1, mo], FP32)
    b_n0 = din("b_n0", [S, 1, no], FP32)
    b_p1b1 = din("b_p1b1", [S, 1, h_dim], FP32)
    b_p1b2 = din("b_p1b2", [S, 1, mo], FP32)
    b_n1 = din("b_n1", [S, 1, no], FP32)
    b_f = din("b_f", [S, 1, 1], FP32)
    eidx = din("eidx", [128, padn_g // 16], I16)
    destid = din("destid", [128, n_chunks], BF16)
    edist = din("edist", [128, n_chunks], FP32)
    iota32 = din("iota32", [128, D_BLK], BF16)
    escal = din("escal", [128, 4], FP32)
    tc_in = din("tc_in", [1, apc // 128], FP32)
    idin = din("ident", [128, 128], FP32)

    prech_out = nc.dram_tensor("prech", [1, apc], FP32, kind="ExternalOutput").ap()
    charg_out = nc.dram_tensor("charg", [1, apc], FP32, kind="ExternalOutput").ap()

    ntab_loc = [nc.dram_tensor(f"ntl{p}", [apc, no], FP32).ap() for p in range(2)]
    ntab = [nc.dram_tensor(f"ntab{p}", [n_tab, no], FP32, addr_space="Shared").ap()
            for p in range(2)]

    with tile.TileContext(nc) as tc:
        import contextlib
        with contextlib.ExitStack() as ctx:
            persist = ctx.enter_context(tc.tile_pool(name="persist", bufs=1))
            mlp = ctx.enter_context(tc.tile_pool(name="mlp", bufs=2))
            xmp = ctx.enter_context(tc.tile_pool(name="xmp", bufs=1))
            psA = ctx.enter_context(tc.tile_pool(name="psA", bufs=2, space="PSUM"))
            psT = ctx.enter_context(tc.tile_pool(name="psT", bufs=2, space="PSUM"))
            gat = ctx.enter_context(tc.tile_pool(name="gat", bufs=3))
            mkp = ctx.enter_context(tc.tile_pool(name="mkp", bufs=4))

            def loadp(ap_in, shape, dt, name):
                t = persist.tile(shape, dt, tag=name)
                nc.sync.dma_start(t[:], ap_in)
                return t

            spid_t = loadp(spid_in[:, :], [128, apc], BF16, "spid")
            iota_t = loadp(iota32[:, :], [128, D_BLK], BF16, "iota")
            escal_t = loadp(escal[:, :], [128, 4], FP32, "escal")
            destid_t = loadp(destid[:, :], [128, n_chunks], BF16, "destid")
            edist_t = loadp(edist[:, :], [128, n_chunks], FP32, "edist")
            tc_t = loadp(tc_in[:, :], [1, apc // 128], FP32, "tc")
            ident_t = loadp(idin[:, :], [128, 128], FP32, "ident")

            # simpler: explicit loads
            def wload3(ap_src, k, s_count, m, name):
                # src AP pre-rearranged to [k, s, m]
                t = persist.tile([k, s_count * m], ap_src.dtype, tag=name)
                nc.sync.dma_start(
                    t[:].rearrange("k (s m) -> k s m", s=s_count), ap_src)
                return t

            def wload4(ap_src, k, s_count, c, m, name):
                # src AP pre-rearranged to [k, s, c, m]
                t = persist.tile([k, s_count * c * m], ap_src.dtype, tag=name)
                nc.sync.dma_start(
                    t[:].rearrange("k (s c m) -> k s c m", s=s_count, c=c), ap_src)
                return t

            w1t = [wload3(w_p0w1[:, :, :].rearrange("s k m -> k s m"),
                          f_in, S, h_dim, "w1t0"),
                   wload4(w_p1w1[:, :, :, :].rearrange("s c k m -> k s c m"),
                          128, S, 2, h_dim, "w1t1")]
            w2t = [wload4(w_p0w2[:, :, :, :].rearrange("s c k m -> k s c m"),
                          128, S, 2, mo, "w2t0"),
                   wload4(w_p1w2[:, :, :, :].rearrange("s c k m -> k s c m"),
                          128, S, 2, mo, "w2t1")]
            nwt = [wload3(w_n0[:, :, :].rearrange("s k m -> k s m"),
                          mo, S, no, "nwt0"),
                   wload3(w_n1[:, :, :].rearrange("s k m -> k s m"),
                          mo, S, no, "nwt1")]
            fwt = wload4(w_f[:, :, :, :].rearrange("s c k m -> k s c m"),
                         128, S, 2, 1, "fwt")
            b1t = [wload3(b_p0b1[:, :, :].rearrange("s k m -> k s m"), 1, S, h_dim, "b1t0"),
                   wload3(b_p1b1[:, :, :].rearrange("s k m -> k s m"), 1, S, h_dim, "b1t1")]
            b2t = [wload3(b_p0b2[:, :, :].rearrange("s k m -> k s m"), 1, S, mo, "b2t0"),
                   wload3(b_p1b2[:, :, :].rearrange("s k m -> k s m"), 1, S, mo, "b2t1")]
            nbt = [wload3(b_n0[:, :, :].rearrange("s k m -> k s m"), 1, S, no, "nbt0"),
                   wload3(b_n1[:, :, :].rearrange("s k m -> k s m"), 1, S, no, "nbt1")]
            fbt = wload3(b_f[:, :, :].rearrange("s k m -> k s m"), 1, S, 1, "fbt")

            # edge weights
            wtile = persist.tile([128, n_chunks], BF16, tag="wtile")
            dec = mlp.tile([128, n_chunks], FP32, tag="dec", bufs=1)
            nc.scalar.activation(dec[:], edist_t[:], AF.Exp,
                                 bias=escal_t[:, 1:2], scale=escal_t[:, 0:1])
            cut = mlp.tile([128, n_chunks], FP32, tag="cut", bufs=1)
            nc.scalar.activation(cut[:], edist_t[:], AF.Sin,
                                 bias=escal_t[:, 2:3], scale=math.pi / CUTOFF)
            nc.vector.tensor_scalar_add(cut[:], cut[:], 1.0)
            nc.vector.tensor_tensor(wtile[:], dec[:], cut[:], AluOpType.mult)

            internT = [persist.tile([mo, apc], FP32, tag=f"internT{p}",
                                    name=f"internT{p}")
                       for p in range(2)]
            mergedT = persist.tile([no, apc], FP32, tag="mergedT")

            mskp = ctx.enter_context(tc.tile_pool(name="mskp", bufs=1))
            idxp = ctx.enter_context(tc.tile_pool(name="idxp", bufs=8))
            _cur_msk = {}

            def build_masks(ct):
                for s in range(S):
                    mt = mskp.tile([128, CT], FP32, tag=f"msk{s}", name=f"msk{s}")
                    nc.vector.tensor_scalar(
                        mt[:], spid_t[:, ct * CT:ct * CT + CT], float(s), None,
                        AluOpType.is_equal)
                    _cur_msk[s] = mt

            def msl(s, ct):
                return _cur_msk[s][:]

            def msl0(s, ct):
                return _cur_msk[s][0:1, :]

            def mlp_phase(p):
                kck = 1 if p == 0 else 2
                for ct in range(ncts):
                    asl = slice(ct * CT, ct * CT + CT)
                    build_masks(ct)
                    if p == 0:
                        xseg = mlp.tile([128, CT], FP32, tag="xseg")
                        nc.sync.dma_start(xseg[:], xT_in[:, asl])
                    # masked inputs for this column tile
                    xms = {}
                    for s in range(S):
                        for kc in range(kck):
                            xm = xmp.tile([128, CT], FP32, tag=f"xm{s}_{kc}")
                            src = (xseg if p == 0
                                   else (internT[0] if kc == 0 else mergedT))
                            nc.vector.tensor_tensor(
                                xm[:], (src[:] if p == 0 else src[:, asl]),
                                msl(s, ct), AluOpType.mult)
                            xms[s, kc] = xm
                    hts = []
                    for hc in range(hck):
                        z1 = psA.tile([128, CT], FP32, tag="zz")
                        for s in range(S):
                            nc.tensor.matmul(
                                z1[:],
                                b1t[p][0:1, s * h_dim + hc * 128:
                                       s * h_dim + hc * 128 + 128],
                                msl0(s, ct), start=(s == 0), stop=False)
                        for s in range(S):
                            for kc in range(kck):
                                if p == 0:
                                    lhsT = w1t[0][:, s * h_dim + hc * 128:
                                                  s * h_dim + hc * 128 + 128]
                                else:
                                    base = s * 2 * h_dim + kc * h_dim + hc * 128
                                    lhsT = w1t[1][:, base:base + 128]
                                nc.tensor.matmul(
                                    z1[:], lhsT,
                                    xms[s, kc][:],
                                    start=False,
                                    stop=(s == S - 1 and kc == kck - 1))
                        mn = mlp.tile([128, CT], FP32, tag="mn")
                        nc.vector.tensor_scalar_min(mn[:], z1[:], 0.0)
                        ex = mlp.tile([128, CT], FP32, tag="ex")
                        nc.scalar.activation(ex[:], mn[:], AF.Exp)
                        ht = mlp.tile([128, CT], FP32, tag="ht")
                        nc.vector.tensor_scalar_max(ht[:], z1[:], 0.0)
                        nc.vector.tensor_tensor(ht[:], ht[:], ex[:], AluOpType.add)
                        hts.append(ht)
                    z2 = psA.tile([128, CT], FP32, tag="zz")
                    for s in range(S):
                        nc.tensor.matmul(
                            z2[:], b2t[p][0:1, s * mo:s * mo + 128],
                            msl0(s, ct), start=(s == 0), stop=False)
                    for s in range(S):
                        for hc in range(hck):
                            hm = mlp.tile([128, CT], FP32, tag="hm")
                            nc.vector.tensor_tensor(
                                hm[:], hts[hc][:], msl(s, ct), AluOpType.mult)
                            base = s * 2 * mo + hc * mo
                            nc.tensor.matmul(
                                z2[:], w2t[p][:, base:base + 128],
                                hm[:],
                                start=False, stop=(s == S - 1 and hc == hck - 1))
                    nc.vector.tensor_copy(internT[p][:, asl], z2[:])
                    zn = psA.tile([128, CT], FP32, tag="zz")
                    for s in range(S):
                        nc.tensor.matmul(
                            zn[:], nbt[p][0:1, s * no:s * no + 128],
                            msl0(s, ct), start=(s == 0), stop=False)
                    for s in range(S):
                        im = mlp.tile([128, CT], FP32, tag="im")
                        nc.vector.tensor_tensor(
                            im[:], internT[p][:, asl], msl(s, ct), AluOpType.mult)
                        nc.tensor.matmul(
                            zn[:], nwt[p][:, s * no:s * no + 128],
                            im[:],
                            start=False, stop=(s == S - 1))
                    nT = mlp.tile([128, CT], FP32, tag="nT")
                    nc.vector.tensor_copy(nT[:], zn[:])
                    for q in range(CT // 128):
                        pt = psT.tile([128, 128], FP32, tag="pt")
                        nc.tensor.transpose(pt[:], nT[:, q * 128:q * 128 + 128],
                                            ident_t[:])
                        rowt = mlp.tile([128, 128], FP32, tag="rowt")
                        nc.vector.tensor_copy(rowt[:], pt[:])
                        r0 = ct * CT + q * 128
                        nc.sync.dma_start(ntab_loc[p][r0:r0 + 128, :], rowt[:])

            def edge_phase(p):
                pscols = 512
                bpp = pscols // D_BLK       # blocks per psum tile (16)
                psm = None
                for gi in range(ngi):
                    it = idxp.tile([128, GSUB * 8], I16, tag="it")
                    nc.sync.dma_start(it[:], eidx[:, gi * GSUB * 8:(gi + 1) * GSUB * 8])
                    gt = gat.tile([128, GSUB, no], FP32, tag="gt")
                    nc.gpsimd.dma_gather(
                        gt[:], ntab[p][:, :], it[:],
                        num_idxs=GSUB * 128, num_idxs_reg=GSUB * 128,
                        elem_size=no)
                    c0 = gi * GSUB
                    nsub = min(GSUB, n_chunks - c0)
                    if nsub <= 0:
                        continue
                    csl = slice(c0, c0 + nsub)
                    mk = mkp.tile([128, GSUB * D_BLK], BF16, tag="mk")
                    nc.vector.tensor_tensor(
                        mk[:, :nsub * D_BLK].rearrange("p (c d) -> p c d", d=D_BLK),
                        destid_t[:, csl].broadcast_to([128, nsub, D_BLK]),
                        iota_t[:].rearrange("p (x d) -> p x d", x=1)
                              .broadcast_to([128, nsub, D_BLK]),
                        AluOpType.is_equal)
                    wm = mkp.tile([128, GSUB * D_BLK], FP32, tag="wm")
                    nc.vector.tensor_tensor(
                        wm[:, :nsub * D_BLK].rearrange("p (c d) -> p c d", d=D_BLK),
                        mk[:, :nsub * D_BLK].rearrange("p (c d) -> p c d", d=D_BLK),
                        wtile[:, csl].broadcast_to([128, nsub, D_BLK]),
                        AluOpType.mult)
                    for sub in range(nsub):
                        g = c0 + sub
                        blk = g // c_blk
                        cin = g % c_blk
                        if blk % bpp == 0 and cin == 0:
                            psm = psT.tile([128, pscols], FP32, tag="psm")
                        col0 = (blk % bpp) * D_BLK
                        nc.tensor.matmul(
                            psm[:, col0:col0 + D_BLK],
                            gt[:, sub, :],
                            wm[:, sub * D_BLK:sub * D_BLK + D_BLK],
                            start=(cin == 0), stop=(cin == c_blk - 1))
                        if cin == c_blk - 1 and blk % bpp == bpp - 1:
                            grp = blk // bpp
                            nc.vector.tensor_copy(
                                mergedT[:, grp * pscols:(grp + 1) * pscols],
                                psm[:])

            def final_phase():
                prech = persist.tile([1, apc], FP32, tag="prech")
                for ct in range(ncts):
                    build_masks(ct)
                    zf = psT.tile([1, CT], FP32, tag="psm")
                    for s in range(S):
                        nc.tensor.matmul(
                            zf[:], fbt[0:1, s:s + 1], msl0(s, ct),
                            start=(s == 0), stop=False)
                    for s in range(S):
                        for kc in range(2):
                            src = internT[1] if kc == 0 else mergedT
                            xm = mlp.tile([128, CT], FP32, tag="xmf")
                            nc.vector.tensor_tensor(
                                xm[:], src[:, ct * CT:ct * CT + CT],
                                msl(s, ct), AluOpType.mult)
                            nc.tensor.matmul(
                                zf[:], fwt[:, s * 2 + kc:s * 2 + kc + 1],
                                xm[:],
                                start=False,
                                stop=(s == S - 1 and kc == 1))
                    nc.vector.tensor_copy(prech[0:1, ct * CT:ct * CT + CT], zf[:])
                nc.sync.dma_start(prech_out[:, :], prech[:])
                nmol = apc // 128
                tpre = mlp.tile([1, nmol], FP32, tag="tpre")
                nc.vector.tensor_reduce(
                    tpre[0:1, :],
                    prech[0:1, :].rearrange("p (m a) -> p m a", a=128),
                    mybir.AxisListType.X, AluOpType.add)
                corr = mlp.tile([1, nmol], FP32, tag="corr")
                nc.vector.tensor_tensor(corr[:], tc_t[:], tpre[:],
                                        AluOpType.subtract)
                nc.vector.tensor_scalar_mul(corr[:], corr[:], 1.0 / 128.0)
                nc.vector.tensor_tensor(
                    prech[0:1, :].rearrange("p (m a) -> p m a", a=128),
                    prech[0:1, :].rearrange("p (m a) -> p m a", a=128),
                    corr[0:1, :].broadcast_to([1, nmol, 128]),
                    AluOpType.add)
                nc.sync.dma_start(charg_out[:, :], prech[:])

            for p in range(2):
                mlp_phase(p)
                nc.gpsimd.collective_compute(
                    "AllGather", AluOpType.bypass,
                    replica_groups=[list(range(N_CORES))],
                    ins=[ntab_loc[p]], outs=[ntab[p]])
                edge_phase(p)
            final_phase()

    nc.compile()
    split_multi_waits(nc)
    return nc


# ---------------------------------------------------------------- host prep
def _wrap_idx(flat_idx):
    n = len(flat_idx)
    a = np.zeros((16, (n + 15) // 16), np.int16)
    a[np.arange(n) % 16, np.arange(n) // 16] = flat_idx
    return np.tile(a, (8, 1))


def prepare_inputs(species, in_features, atom_index12, distances, total_charges,
                   p0_w1, p0_b1, p0_w2, p0_b2, n0_w, n0_b,
                   p1_w1, p1_b1, p1_w2, p1_b2, n1_w, n1_b,
                   f_w, f_b, prefactor, factor):
    B, A = np.asarray(species).shape
    N = B * A
    F_IN = np.asarray(in_features).shape[-1]
    H = np.asarray(p0_w1).shape[-1]
    MO = np.asarray(p0_w2).shape[-1]
    NO = np.asarray(n0_w).shape[-1]
    APC = N // N_CORES
    sp = np.asarray(species).reshape(-1).astype(np.int64)
    feats = np.asarray(in_features, np.float32).reshape(N, F_IN)

    # atoms keep their natural order (core c owns [c*APC, (c+1)*APC));
    # species routing is fully mask-based so no sorting is needed, and
    # molecule boundaries (128-atom groups) stay intact for the charge
    # correction.
    perm = np.arange(N)
    inv = perm
    sp_sorted = sp

    i0 = inv[np.asarray(atom_index12[0], np.int64)]
    i1 = inv[np.asarray(atom_index12[1], np.int64)]
    dd = np.asarray(distances, np.float32)
    dest = np.concatenate([i0, i1])
    src = np.concatenate([i1, i0])
    ddist = np.concatenate([dd, dd])

    nblocks = APC // D_BLK
    dcore = dest // APC
    dloc = dest - dcore * APC
    dblk = dloc // D_BLK

    counts = np.bincount(dcore * nblocks + dblk, minlength=N_CORES * nblocks)
    c_blk = int(np.ceil(counts.max() / 128.0))
    n_chunks = nblocks * c_blk
    ngi = (n_chunks + GSUB - 1) // GSUB
    padn_g = ngi * GSUB * 128
    slots = n_chunks * 128

    key = dcore * nblocks + dblk
    order = np.argsort(key, kind="stable")
    bounds = np.searchsorted(key[order], np.arange(N_CORES * nblocks + 1))

    eidx_np = np.zeros((N_CORES, 128, padn_g // 16), np.int16)
    destid_np = np.zeros((N_CORES, 128, n_chunks), ml_dtypes.bfloat16)
    edist_np = np.zeros((N_CORES, 128, n_chunks), np.float32)
    j = np.arange(slots)
    for c in range(N_CORES):
        idx_flat = np.zeros(slots, np.int64)
        did_flat = np.full(slots, float(D_BLK), np.float32)   # pad -> no match
        dst_flat = np.zeros(slots, np.float32)
        for b in range(nblocks):
            g0, g1 = bounds[c * nblocks + b], bounds[c * nblocks + b + 1]
            cnt = g1 - g0
            s0 = b * c_blk * 128
            sel = order[g0:g1]
            idx_flat[s0:s0 + cnt] = src[sel]
            did_flat[s0:s0 + cnt] = (dloc[sel] % D_BLK).astype(np.float32)
            dst_flat[s0:s0 + cnt] = ddist[sel]
        eidx_np[c] = _wrap_idx(np.concatenate(
            [idx_flat, np.zeros(padn_g - slots, np.int64)]).astype(np.int16))
        destid_np[c, j % 128, j // 128] = did_flat.astype(ml_dtypes.bfloat16)
        edist_np[c, j % 128, j // 128] = dst_flat

    def bf(x):
        return np.ascontiguousarray(np.asarray(x, np.float32)).astype(
            ml_dtypes.bfloat16)

    def f32(x):
        return np.ascontiguousarray(np.asarray(x, np.float32))

    pf = float(np.asarray(prefactor)); fc = float(np.asarray(factor))
    escal_np = np.zeros((128, 4), np.float32)
    escal_np[:, 2] = math.pi / 2.0
    escal_np[:, 0] = -fc * fc
    escal_np[:, 1] = math.log(max(0.5 * pf * pf, 1e-30))

    p0b2_adj = np.asarray(p0_b2, np.float64) - np.asarray(p0_w2, np.float64).sum(1)
    p1b2_adj = np.asarray(p1_b2, np.float64) - np.asarray(p1_w2, np.float64).sum(1)

    def kchunk(w):  # [S, 2k, m] -> [S, 2, 128, m]
        w = np.asarray(w, np.float32)
        return w.reshape(w.shape[0], 2, 128, w.shape[-1])

    common = {
        "w_p0w1": f32(p0_w1), "w_p0w2": f32(kchunk(p0_w2)), "w_n0": f32(n0_w),
        "w_p1w1": f32(kchunk(p1_w1)), "w_p1w2": f32(kchunk(p1_w2)),
        "w_n1": f32(n1_w), "w_f": f32(kchunk(f_w)),
        "b_p0b1": f32(np.asarray(p0_b1))[:, None, :],
        "b_p0b2": f32(p0b2_adj)[:, None, :],
        "b_n0": f32(np.asarray(n0_b))[:, None, :],
        "b_p1b1": f32(np.asarray(p1_b1))[:, None, :],
        "b_p1b2": f32(p1b2_adj)[:, None, :],
        "b_n1": f32(np.asarray(n1_b))[:, None, :],
        "b_f": f32(np.asarray(f_b))[:, None, :],
        "iota32": np.tile(np.arange(D_BLK, dtype=np.float32).astype(
            ml_dtypes.bfloat16)[None, :], (128, 1)),
        "escal": escal_np,
        "ident": np.eye(128, dtype=np.float32),
    }

    tc_np = np.asarray(total_charges, np.float32)
    in_maps = []
    for c in range(N_CORES):
        asl = slice(c * APC, (c + 1) * APC)
        spc = sp_sorted[asl]
        xT = np.ascontiguousarray(feats[perm[asl]].T)
        spid_c = np.tile(spc.astype(np.float32)[None, :], (128, 1)).astype(
            ml_dtypes.bfloat16)
        in_maps.append({
            "xT": xT, "spid": spid_c,
            "eidx": eidx_np[c], "destid": destid_np[c], "edist": edist_np[c],
            "tc_in": tc_np[c * (B // N_CORES):(c + 1) * (B // N_CORES)][None, :],
            **common,
        })
    meta = dict(perm=perm, B=B, A=A, APC=APC, c_blk=c_blk,
                F_IN=F_IN, H=H, MO=MO, NO=NO)
    return in_maps, meta


# ---------------------------------------------------------------- runner
class SpmdRunner:
    def __init__(self, nc, n_cores=N_CORES):
        import jax
        from concourse import bass2jax
        from concourse.bass2jax import _bass_exec_p, install_neuronx_cc_hook
        from jax.sharding import Mesh, PartitionSpec
        from jax.experimental.shard_map import shard_map
        install_neuronx_cc_hook()
        self.jax = jax
        self.nc = nc
        self.n_cores = n_cores
        in_names, out_names, out_avals, zero_outs = [], [], [], []
        partition_name = (nc.partition_id_tensor.name
                          if nc.partition_id_tensor else None)
        for alloc in nc.m.functions[0].allocations:
            if not isinstance(alloc, mybir.MemoryLocationSet):
                continue
            name = alloc.memorylocations[0].name
            if alloc.kind == "ExternalInput":
                if name != partition_name:
                    in_names.append(name)
            elif alloc.kind == "ExternalOutput":
                shape = tuple(alloc.tensor_shape)
                dtype = mybir.dt.np(alloc.dtype)
                out_names.append(name)
                out_avals.append(jax.core.ShapedArray(shape, dtype))
                zero_outs.append(np.zeros(shape, dtype))
        n_params = len(in_names)
        all_in = in_names + out_names
        if partition_name is not None:
            all_in.append(partition_name)

        def _body(*args):
            operands = list(args)
            if partition_name is not None:
                operands.append(bass2jax.partition_id_tensor())
            outs = _bass_exec_p.bind(
                *operands, out_avals=tuple(out_avals), in_names=tuple(all_in),
                out_names=tuple(out_names), lowering_input_output_aliases=(),
                sim_require_finite=True, sim_require_nnan=True, nc=nc)
            return tuple(outs)

        devices = jax.devices()[:n_cores]
        mesh = Mesh(np.asarray(devices), ("core",))
        in_specs = (PartitionSpec("core"),) * (n_params + len(out_names))
        out_specs = (PartitionSpec("core"),) * len(out_names)
        self._fn = jax.jit(
            shard_map(_body, mesh=mesh, in_specs=in_specs,
                      out_specs=out_specs, check_rep=False),
            keep_unused=True)
        self.in_names, self.out_names = in_names, out_names
        self.out_avals, self.zero_outs = out_avals, zero_outs
        self.n_params = n_params

    def prepare(self, in_maps):
        per_core = [[np.asarray(m[n]) for n in self.in_names] for m in in_maps]
        concat_in = [
            np.concatenate([per_core[c][i] for c in range(self.n_cores)], axis=0)
            for i in range(self.n_params)]
        concat_zeros = [
            np.zeros((self.n_cores * z.shape[0], *z.shape[1:]), z.dtype)
            for z in self.zero_outs]
        args = [self.jax.device_put(a) for a in concat_in + concat_zeros]
        for a in args:
            a.block_until_ready()
        self._args = args

    def run(self):
        outs = self._fn(*self._args)
        self.jax.block_until_ready(outs)
        return outs

    def run_async(self):
        return self._fn(*self._args)

    def results(self, outs):
        return [
            {name: np.asarray(outs[i]).reshape(
                self.n_cores, *self.out_avals[i].shape)[c]
             for i, name in enumerate(self.out_names)}
            for c in range(self.n_cores)]


def jax_block(x):
    import jax
    jax.block_until_ready(x)


_CACHE = {}


def _get_runner(apc, f_in, h_dim, mo, no, c_blk):
    key = (apc, f_in, h_dim, mo, no, c_blk)
    if key not in _CACHE:
        nc = build_program(apc, f_in, h_dim, mo, no, c_blk)
        _CACHE[key] = SpmdRunner(nc, N_CORES)
    return _CACHE[key]


def kernel(**inputs):
    species = inputs["species"]
    in_maps, meta = prepare_inputs(**inputs)
    r = _get_runner(meta["APC"], meta["F_IN"], meta["H"], meta["MO"],
                    meta["NO"], meta["c_blk"])
    r.prepare(in_maps)
    outs = r.run()
    res = r.results(outs)
    N = meta["B"] * meta["A"]
    prech = np.empty(N, np.float32)
    charg = np.empty(N, np.float32)
    for c in range(N_CORES):
        asl = slice(c * meta["APC"], (c + 1) * meta["APC"])
        prech[meta["perm"][asl]] = res[c]["prech"][0]
        charg[meta["perm"][asl]] = res[c]["charg"][0]
    B, A = meta["B"], meta["A"]
    return species, charg.reshape(B, A), prech.reshape(B, A)



# revision 7
# speedup vs baseline: 3.2433x; 1.5503x over previous
"""Trainium2 Bass kernel for nn_LocalMessagePassing (2-pass GNN message passing).

8 NeuronCores, SPMD, data-parallel over molecules (4096 atoms/core):
- species routing via masked PSUM accumulation (4 species matmuls + K=1
  outer-product bias matmuls into the same accumulator)
- celu(z) = max(z,0)+exp(min(z,0))-1, the -1 folded into next-layer bias
- per pass: slice MLP -> bf16 neigh table -> AllGather -> dma_gather of edge
  contributions (dest-sorted, per-32-dest-block padded) -> segment-sum via
  matmul (mergedT += gathered^T @ (onehot*edge_w)) accumulated in PSUM
- final routed linear (M=1 matmuls) + per-molecule charge correction on chip
"""
import sys
sys.path.insert(0, "/opt/trn_rl_repo")
import math
import numpy as np
import ml_dtypes

import concourse.bacc as bacc
import concourse.mybir as mybir
import concourse.tile as tile
from concourse.alu_op_type import AluOpType

BF16 = mybir.dt.bfloat16
FP32 = mybir.dt.float32
I16 = mybir.dt.int16
AF = mybir.ActivationFunctionType
F32R = mybir.dt.float32r

N_CORES = 8
S = 4
CUTOFF = 5.2
D_BLK = 32
GSUB = 7            # 128-idx chunks per dma_gather (57 ring entries)


def split_multi_waits(nc):
    """This walrus build allows one sync-wait per instruction; hoist extras
    onto same-engine NOPs placed immediately before."""
    cnt = 0
    for fn in nc.m.functions:
        for bb in fn.blocks:
            out = []
            changed = False
            for ins in bb.instructions:
                si = ins.sync_info
                if si is not None and len(si.on_wait) > 1:
                    waits = list(si.on_wait)
                    for w in waits[:-1]:
                        cnt += 1
                        out.append(mybir.InstNoOp(
                            name=f"wsplit-{cnt}", engine=ins.engine,
                            bass_nofuse=True,
                            sync_info=mybir.SyncInfo(on_wait=[w], on_update=[]),
                        ))
                    ins.sync_info = mybir.SyncInfo(
                        on_wait=[waits[-1]], on_update=list(si.on_update))
                    changed = True
                out.append(ins)
            if changed:
                bb.instructions = out
    return cnt


# ------------------------------------------------------------- blob layout
# All per-core inputs are packed into ONE f32 dram tensor; per-operand
# overhead of the tunneled PJRT execute (~1.6 ms/operand) dominated the
# baseline runtime.
def blob_layout(apc, f_in, h_dim, mo, no, c_blk):
    nblocks = apc // D_BLK
    n_chunks = nblocks * c_blk
    ngi = (n_chunks + GSUB - 1) // GSUB
    padn_g = ngi * GSUB * 128
    secs = [
        ("xT", (f_in, apc), np.float32),
        ("spid", (128, apc), ml_dtypes.bfloat16),
        ("w_p0w1", (S, f_in, h_dim), np.float32),
        ("w_p0w2", (S, 2, 128, mo), np.float32),
        ("w_n0", (S, mo, no), np.float32),
        ("w_p1w1", (S, 2, 128, h_dim), np.float32),
        ("w_p1w2", (S, 2, 128, mo), np.float32),
        ("w_n1", (S, mo, no), np.float32),
        ("w_f", (S, 2, 128, 1), np.float32),
        ("b_p0b1", (S, 1, h_dim), np.float32),
        ("b_p0b2", (S, 1, mo), np.float32),
        ("b_n0", (S, 1, no), np.float32),
        ("b_p1b1", (S, 1, h_dim), np.float32),
        ("b_p1b2", (S, 1, mo), np.float32),
        ("b_n1", (S, 1, no), np.float32),
        ("b_f", (S, 1, 1), np.float32),
        ("eidx", (128, padn_g // 16), np.int16),
        ("destid", (128, n_chunks), ml_dtypes.bfloat16),
        ("edist", (128, n_chunks), np.float32),
        ("iota32", (128, D_BLK), ml_dtypes.bfloat16),
        ("escal", (128, 4), np.float32),
        ("tc_in", (1, apc // 128), np.float32),
        ("ident", (128, 128), np.float32),
    ]
    layout = {}
    off = 0
    for name, shape, dt in secs:
        n_elem = int(np.prod(shape))
        n_f32 = n_elem * np.dtype(dt).itemsize // 4
        layout[name] = (off, shape, dt)
        off += (n_f32 + 15) // 16 * 16
    return layout, off


# ---------------------------------------------------------------- program
def build_program(apc, f_in, h_dim, mo, no, c_blk):
    nblocks = apc // D_BLK
    n_chunks = nblocks * c_blk
    ngi = (n_chunks + GSUB - 1) // GSUB
    padn_g = ngi * GSUB * 128
    n_tab = N_CORES * apc
    CT = 512
    ncts = apc // CT
    hck = h_dim // 128

    nc = bacc.Bacc("TRN2", target_bir_lowering=False, debug=False,
                   num_devices=N_CORES)

    layout, tot = blob_layout(apc, f_in, h_dim, mo, no, c_blk)
    blob = nc.dram_tensor("blob", [tot], FP32, kind="ExternalInput").ap()
    _mydt = {np.float32: FP32, ml_dtypes.bfloat16: BF16, np.int16: I16}

    def din(name, shape, dt):
        off, lshape, ldt = layout[name]
        assert tuple(shape) == tuple(lshape) and _mydt[ldt] == dt
        n_elem = int(np.prod(shape))
        n_f32 = n_elem * np.dtype(ldt).itemsize // 4
        ap = blob[off:off + n_f32]
        if dt != FP32:
            ap = ap.bitcast(dt)
        dims = " ".join(f"d{i}" for i in range(len(shape)))
        return ap.rearrange(f"({dims}) -> {dims}",
                            **{f"d{i}": s for i, s in enumerate(shape)})

    xT_in = din("xT", [f_in, apc], FP32)
    spid_in = din("spid", [128, apc], BF16)
    w_p0w1 = din("w_p0w1", [S, f_in, h_dim], FP32)
    w_p0w2 = din("w_p0w2", [S, 2, 128, mo], FP32)
    w_n0 = din("w_n0", [S, mo, no], FP32)
    w_p1w1 = din("w_p1w1", [S, 2, 128, h_dim], FP32)
    w_p1w2 = din("w_p1w2", [S, 2, 128, mo], FP32)
    w_n1 = din("w_n1", [S, mo, no], FP32)
    w_f = din("w_f", [S, 2, 128, 1], FP32)
    b_p0b1 = din("b_p0b1", [S, 1, h_dim], FP32)
    b_p0b2 = din("b_p0b2", [S, 1, mo], FP32)
    b_n0 = din("b_n0", [S, 1, no], FP32)
    b_p1b1 = din("b_p1b1", [S, 1, h_dim], FP32)
    b_p1b2 = din("b_p1b2", [S, 1, mo], FP32)
    b_n1 = din("b_n1", [S, 1, no], FP32)
    b_f = din("b_f", [S, 1, 1], FP32)
    eidx = din("eidx", [128, padn_g // 16], I16)
    destid = din("destid", [128, n_chunks], BF16)
    edist = din("edist", [128, n_chunks], FP32)
    iota32 = din("iota32", [128, D_BLK], BF16)
    escal = din("escal", [128, 4], FP32)
    tc_in = din("tc_in", [1, apc // 128], FP32)
    idin = din("ident", [128, 128], FP32)

    out_t = nc.dram_tensor("out", [2, apc], FP32, kind="ExternalOutput").ap()
    prech_out = out_t[0:1, :]
    charg_out = out_t[1:2, :]

    ntab_loc = [nc.dram_tensor(f"ntl{p}", [apc, no], FP32).ap() for p in range(2)]
    ntab = [nc.dram_tensor(f"ntab{p}", [n_tab, no], FP32, addr_space="Shared").ap()
            for p in range(2)]

    with tile.TileContext(nc) as tc:
        import contextlib
        with contextlib.ExitStack() as ctx:
            persist = ctx.enter_context(tc.tile_pool(name="persist", bufs=1))
            mlp = ctx.enter_context(tc.tile_pool(name="mlp", bufs=2))
            xmp = ctx.enter_context(tc.tile_pool(name="xmp", bufs=1))
            psA = ctx.enter_context(tc.tile_pool(name="psA", bufs=2, space="PSUM"))
            psT = ctx.enter_context(tc.tile_pool(name="psT", bufs=2, space="PSUM"))
            gat = ctx.enter_context(tc.tile_pool(name="gat", bufs=3))
            mkp = ctx.enter_context(tc.tile_pool(name="mkp", bufs=4))

            def loadp(ap_in, shape, dt, name):
                t = persist.tile(shape, dt, tag=name)
                nc.sync.dma_start(t[:], ap_in)
                return t

            spid_t = loadp(spid_in[:, :], [128, apc], BF16, "spid")
            iota_t = loadp(iota32[:, :], [128, D_BLK], BF16, "iota")
            escal_t = loadp(escal[:, :], [128, 4], FP32, "escal")
            destid_t = loadp(destid[:, :], [128, n_chunks], BF16, "destid")
            edist_t = loadp(edist[:, :], [128, n_chunks], FP32, "edist")
            tc_t = loadp(tc_in[:, :], [1, apc // 128], FP32, "tc")
            ident_t = loadp(idin[:, :], [128, 128], FP32, "ident")

            # simpler: explicit loads
            def wload3(ap_src, k, s_count, m, name):
                # src AP pre-rearranged to [k, s, m]
                t = persist.tile([k, s_count * m], ap_src.dtype, tag=name)
                nc.sync.dma_start(
                    t[:].rearrange("k (s m) -> k s m", s=s_count), ap_src)
                return t

            def wload4(ap_src, k, s_count, c, m, name):
                # src AP pre-rearranged to [k, s, c, m]
                t = persist.tile([k, s_count * c * m], ap_src.dtype, tag=name)
                nc.sync.dma_start(
                    t[:].rearrange("k (s c m) -> k s c m", s=s_count, c=c), ap_src)
                return t

            w1t = [wload3(w_p0w1[:, :, :].rearrange("s k m -> k s m"),
                          f_in, S, h_dim, "w1t0"),
                   wload4(w_p1w1[:, :, :, :].rearrange("s c k m -> k s c m"),
                          128, S, 2, h_dim, "w1t1")]
            w2t = [wload4(w_p0w2[:, :, :, :].rearrange("s c k m -> k s c m"),
                          128, S, 2, mo, "w2t0"),
                   wload4(w_p1w2[:, :, :, :].rearrange("s c k m -> k s c m"),
                          128, S, 2, mo, "w2t1")]
            nwt = [wload3(w_n0[:, :, :].rearrange("s k m -> k s m"),
                          mo, S, no, "nwt0"),
                   wload3(w_n1[:, :, :].rearrange("s k m -> k s m"),
                          mo, S, no, "nwt1")]
            fwt = wload4(w_f[:, :, :, :].rearrange("s c k m -> k s c m"),
                         128, S, 2, 1, "fwt")
            b1t = [wload3(b_p0b1[:, :, :].rearrange("s k m -> k s m"), 1, S, h_dim, "b1t0"),
                   wload3(b_p1b1[:, :, :].rearrange("s k m -> k s m"), 1, S, h_dim, "b1t1")]
            b2t = [wload3(b_p0b2[:, :, :].rearrange("s k m -> k s m"), 1, S, mo, "b2t0"),
                   wload3(b_p1b2[:, :, :].rearrange("s k m -> k s m"), 1, S, mo, "b2t1")]
            nbt = [wload3(b_n0[:, :, :].rearrange("s k m -> k s m"), 1, S, no, "nbt0"),
                   wload3(b_n1[:, :, :].rearrange("s k m -> k s m"), 1, S, no, "nbt1")]
            fbt = wload3(b_f[:, :, :].rearrange("s k m -> k s m"), 1, S, 1, "fbt")

            # edge weights
            wtile = persist.tile([128, n_chunks], BF16, tag="wtile")
            dec = mlp.tile([128, n_chunks], FP32, tag="dec", bufs=1)
            nc.scalar.activation(dec[:], edist_t[:], AF.Exp,
                                 bias=escal_t[:, 1:2], scale=escal_t[:, 0:1])
            cut = mlp.tile([128, n_chunks], FP32, tag="cut", bufs=1)
            nc.scalar.activation(cut[:], edist_t[:], AF.Sin,
                                 bias=escal_t[:, 2:3], scale=math.pi / CUTOFF)
            nc.vector.tensor_scalar_add(cut[:], cut[:], 1.0)
            nc.vector.tensor_tensor(wtile[:], dec[:], cut[:], AluOpType.mult)

            internT = [persist.tile([mo, apc], FP32, tag=f"internT{p}",
                                    name=f"internT{p}")
                       for p in range(2)]
            mergedT = persist.tile([no, apc], FP32, tag="mergedT")

            mskp = ctx.enter_context(tc.tile_pool(name="mskp", bufs=1))
            idxp = ctx.enter_context(tc.tile_pool(name="idxp", bufs=8))
            _cur_msk = {}

            def build_masks(ct):
                for s in range(S):
                    mt = mskp.tile([128, CT], FP32, tag=f"msk{s}", name=f"msk{s}")
                    nc.vector.tensor_scalar(
                        mt[:], spid_t[:, ct * CT:ct * CT + CT], float(s), None,
                        AluOpType.is_equal)
                    _cur_msk[s] = mt

            def msl(s, ct):
                return _cur_msk[s][:]

            def msl0(s, ct):
                return _cur_msk[s][0:1, :]

            def mlp_phase(p):
                kck = 1 if p == 0 else 2
                for ct in range(ncts):
                    asl = slice(ct * CT, ct * CT + CT)
                    build_masks(ct)
                    if p == 0:
                        xseg = mlp.tile([128, CT], FP32, tag="xseg")
                        nc.sync.dma_start(xseg[:], xT_in[:, asl])
                    # masked inputs for this column tile
                    xms = {}
                    for s in range(S):
                        for kc in range(kck):
                            xm = xmp.tile([128, CT], FP32, tag=f"xm{s}_{kc}")
                            src = (xseg if p == 0
                                   else (internT[0] if kc == 0 else mergedT))
                            nc.vector.tensor_tensor(
                                xm[:], (src[:] if p == 0 else src[:, asl]),
                                msl(s, ct), AluOpType.mult)
                            xms[s, kc] = xm
                    hts = []
                    for hc in range(hck):
                        z1 = psA.tile([128, CT], FP32, tag="zz")
                        for s in range(S):
                            nc.tensor.matmul(
                                z1[:],
                                b1t[p][0:1, s * h_dim + hc * 128:
                                       s * h_dim + hc * 128 + 128],
                                msl0(s, ct), start=(s == 0), stop=False)
                        for s in range(S):
                            for kc in range(kck):
                                if p == 0:
                                    lhsT = w1t[0][:, s * h_dim + hc * 128:
                                                  s * h_dim + hc * 128 + 128]
                                else:
                                    base = s * 2 * h_dim + kc * h_dim + hc * 128
                                    lhsT = w1t[1][:, base:base + 128]
                                nc.tensor.matmul(
                                    z1[:], lhsT,
                                    xms[s, kc][:],
                                    start=False,
                                    stop=(s == S - 1 and kc == kck - 1))
                        mn = mlp.tile([128, CT], FP32, tag="mn")
                        nc.vector.tensor_scalar_min(mn[:], z1[:], 0.0)
                        ex = mlp.tile([128, CT], FP32, tag="ex")
                        nc.scalar.activation(ex[:], mn[:], AF.Exp)
                        ht = mlp.tile([128, CT], FP32, tag="ht")
                        nc.vector.tensor_scalar_max(ht[:], z1[:], 0.0)
                        nc.vector.tensor_tensor(ht[:], ht[:], ex[:], AluOpType.add)
                        hts.append(ht)
                    z2 = psA.tile([128, CT], FP32, tag="zz")
                    for s in range(S):
                        nc.tensor.matmul(
                            z2[:], b2t[p][0:1, s * mo:s * mo + 128],
                            msl0(s, ct), start=(s == 0), stop=False)
                    for s in range(S):
                        for hc in range(hck):
                            hm = mlp.tile([128, CT], FP32, tag="hm")
                            nc.vector.tensor_tensor(
                                hm[:], hts[hc][:], msl(s, ct), AluOpType.mult)
                            base = s * 2 * mo + hc * mo
                            nc.tensor.matmul(
                                z2[:], w2t[p][:, base:base + 128],
                                hm[:],
                                start=False, stop=(s == S - 1 and hc == hck - 1))
                    nc.vector.tensor_copy(internT[p][:, asl], z2[:])
                    zn = psA.tile([128, CT], FP32, tag="zz")
                    for s in range(S):
                        nc.tensor.matmul(
                            zn[:], nbt[p][0:1, s * no:s * no + 128],
                            msl0(s, ct), start=(s == 0), stop=False)
                    for s in range(S):
                        im = mlp.tile([128, CT], FP32, tag="im")
                        nc.vector.tensor_tensor(
                            im[:], internT[p][:, asl], msl(s, ct), AluOpType.mult)
                        nc.tensor.matmul(
                            zn[:], nwt[p][:, s * no:s * no + 128],
                            im[:],
                            start=False, stop=(s == S - 1))
                    nT = mlp.tile([128, CT], FP32, tag="nT")
                    nc.vector.tensor_copy(nT[:], zn[:])
                    for q in range(CT // 128):
                        pt = psT.tile([128, 128], FP32, tag="pt")
                        nc.tensor.transpose(pt[:], nT[:, q * 128:q * 128 + 128],
                                            ident_t[:])
                        rowt = mlp.tile([128, 128], FP32, tag="rowt")
                        nc.vector.tensor_copy(rowt[:], pt[:])
                        r0 = ct * CT + q * 128
                        nc.sync.dma_start(ntab_loc[p][r0:r0 + 128, :], rowt[:])

            def edge_phase(p):
                pscols = 512
                bpp = pscols // D_BLK       # blocks per psum tile (16)
                psm = None
                for gi in range(ngi):
                    it = idxp.tile([128, GSUB * 8], I16, tag="it")
                    nc.sync.dma_start(it[:], eidx[:, gi * GSUB * 8:(gi + 1) * GSUB * 8])
                    gt = gat.tile([128, GSUB, no], FP32, tag="gt")
                    nc.gpsimd.dma_gather(
                        gt[:], ntab[p][:, :], it[:],
                        num_idxs=GSUB * 128, num_idxs_reg=GSUB * 128,
                        elem_size=no)
                    c0 = gi * GSUB
                    nsub = min(GSUB, n_chunks - c0)
                    if nsub <= 0:
                        continue
                    csl = slice(c0, c0 + nsub)
                    mk = mkp.tile([128, GSUB * D_BLK], BF16, tag="mk")
                    nc.vector.tensor_tensor(
                        mk[:, :nsub * D_BLK].rearrange("p (c d) -> p c d", d=D_BLK),
                        destid_t[:, csl].broadcast_to([128, nsub, D_BLK]),
                        iota_t[:].rearrange("p (x d) -> p x d", x=1)
                              .broadcast_to([128, nsub, D_BLK]),
                        AluOpType.is_equal)
                    wm = mkp.tile([128, GSUB * D_BLK], FP32, tag="wm")
                    nc.vector.tensor_tensor(
                        wm[:, :nsub * D_BLK].rearrange("p (c d) -> p c d", d=D_BLK),
                        mk[:, :nsub * D_BLK].rearrange("p (c d) -> p c d", d=D_BLK),
                        wtile[:, csl].broadcast_to([128, nsub, D_BLK]),
                        AluOpType.mult)
                    for sub in range(nsub):
                        g = c0 + sub
                        blk = g // c_blk
                        cin = g % c_blk
                        if blk % bpp == 0 and cin == 0:
                            psm = psT.tile([128, pscols], FP32, tag="psm")
                        col0 = (blk % bpp) * D_BLK
                        nc.tensor.matmul(
                            psm[:, col0:col0 + D_BLK],
                            gt[:, sub, :],
                            wm[:, sub * D_BLK:sub * D_BLK + D_BLK],
                            start=(cin == 0), stop=(cin == c_blk - 1))
                        if cin == c_blk - 1 and blk % bpp == bpp - 1:
                            grp = blk // bpp
                            nc.vector.tensor_copy(
                                mergedT[:, grp * pscols:(grp + 1) * pscols],
                                psm[:])

            def final_phase():
                prech = persist.tile([1, apc], FP32, tag="prech")
                for ct in range(ncts):
                    build_masks(ct)
                    zf = psT.tile([1, CT], FP32, tag="psm")
                    for s in range(S):
                        nc.tensor.matmul(
                            zf[:], fbt[0:1, s:s + 1], msl0(s, ct),
                            start=(s == 0), stop=False)
                    for s in range(S):
                        for kc in range(2):
                            src = internT[1] if kc == 0 else mergedT
                            xm = mlp.tile([128, CT], FP32, tag="xmf")
                            nc.vector.tensor_tensor(
                                xm[:], src[:, ct * CT:ct * CT + CT],
                                msl(s, ct), AluOpType.mult)
                            nc.tensor.matmul(
                                zf[:], fwt[:, s * 2 + kc:s * 2 + kc + 1],
                                xm[:],
                                start=False,
                                stop=(s == S - 1 and kc == 1))
                    nc.vector.tensor_copy(prech[0:1, ct * CT:ct * CT + CT], zf[:])
                nc.sync.dma_start(prech_out[:, :], prech[:])
                nmol = apc // 128
                tpre = mlp.tile([1, nmol], FP32, tag="tpre")
                nc.vector.tensor_reduce(
                    tpre[0:1, :],
                    prech[0:1, :].rearrange("p (m a) -> p m a", a=128),
                    mybir.AxisListType.X, AluOpType.add)
                corr = mlp.tile([1, nmol], FP32, tag="corr")
                nc.vector.tensor_tensor(corr[:], tc_t[:], tpre[:],
                                        AluOpType.subtract)
                nc.vector.tensor_scalar_mul(corr[:], corr[:], 1.0 / 128.0)
                nc.vector.tensor_tensor(
                    prech[0:1, :].rearrange("p (m a) -> p m a", a=128),
                    prech[0:1, :].rearrange("p (m a) -> p m a", a=128),
                    corr[0:1, :].broadcast_to([1, nmol, 128]),
                    AluOpType.add)
                nc.sync.dma_start(charg_out[:, :], prech[:])

            for p in range(2):
                mlp_phase(p)
                nc.gpsimd.collective_compute(
                    "AllGather", AluOpType.bypass,
                    replica_groups=[list(range(N_CORES))],
                    ins=[ntab_loc[p]], outs=[ntab[p]])
                edge_phase(p)
            final_phase()

    nc.compile()
    split_multi_waits(nc)
    return nc


# ---------------------------------------------------------------- host prep
def _wrap_idx(flat_idx):
    n = len(flat_idx)
    a = np.zeros((16, (n + 15) // 16), np.int16)
    a[np.arange(n) % 16, np.arange(n) // 16] = flat_idx
    return np.tile(a, (8, 1))


def prepare_inputs(species, in_features, atom_index12, distances, total_charges,
                   p0_w1, p0_b1, p0_w2, p0_b2, n0_w, n0_b,
                   p1_w1, p1_b1, p1_w2, p1_b2, n1_w, n1_b,
                   f_w, f_b, prefactor, factor):
    B, A = np.asarray(species).shape
    N = B * A
    F_IN = np.asarray(in_features).shape[-1]
    H = np.asarray(p0_w1).shape[-1]
    MO = np.asarray(p0_w2).shape[-1]
    NO = np.asarray(n0_w).shape[-1]
    APC = N // N_CORES
    sp = np.asarray(species).reshape(-1).astype(np.int64)
    feats = np.asarray(in_features, np.float32).reshape(N, F_IN)

    # atoms keep their natural order (core c owns [c*APC, (c+1)*APC));
    # species routing is fully mask-based so no sorting is needed, and
    # molecule boundaries (128-atom groups) stay intact for the charge
    # correction.
    perm = np.arange(N)
    inv = perm
    sp_sorted = sp

    i0 = inv[np.asarray(atom_index12[0], np.int64)]
    i1 = inv[np.asarray(atom_index12[1], np.int64)]
    dd = np.asarray(distances, np.float32)
    dest = np.concatenate([i0, i1])
    src = np.concatenate([i1, i0])
    ddist = np.concatenate([dd, dd])

    nblocks = APC // D_BLK
    dcore = dest // APC
    dloc = dest - dcore * APC
    dblk = dloc // D_BLK

    counts = np.bincount(dcore * nblocks + dblk, minlength=N_CORES * nblocks)
    c_blk = int(np.ceil(counts.max() / 128.0))
    n_chunks = nblocks * c_blk
    ngi = (n_chunks + GSUB - 1) // GSUB
    padn_g = ngi * GSUB * 128
    slots = n_chunks * 128

    key = dcore * nblocks + dblk
    order = np.argsort(key, kind="stable")
    bounds = np.searchsorted(key[order], np.arange(N_CORES * nblocks + 1))

    eidx_np = np.zeros((N_CORES, 128, padn_g // 16), np.int16)
    destid_np = np.zeros((N_CORES, 128, n_chunks), ml_dtypes.bfloat16)
    edist_np = np.zeros((N_CORES, 128, n_chunks), np.float32)
    j = np.arange(slots)
    for c in range(N_CORES):
        idx_flat = np.zeros(slots, np.int64)
        did_flat = np.full(slots, float(D_BLK), np.float32)   # pad -> no match
        dst_flat = np.zeros(slots, np.float32)
        for b in range(nblocks):
            g0, g1 = bounds[c * nblocks + b], bounds[c * nblocks + b + 1]
            cnt = g1 - g0
            s0 = b * c_blk * 128
            sel = order[g0:g1]
            idx_flat[s0:s0 + cnt] = src[sel]
            did_flat[s0:s0 + cnt] = (dloc[sel] % D_BLK).astype(np.float32)
            dst_flat[s0:s0 + cnt] = ddist[sel]
        eidx_np[c] = _wrap_idx(np.concatenate(
            [idx_flat, np.zeros(padn_g - slots, np.int64)]).astype(np.int16))
        destid_np[c, j % 128, j // 128] = did_flat.astype(ml_dtypes.bfloat16)
        edist_np[c, j % 128, j // 128] = dst_flat

    def f32(x):
        return np.ascontiguousarray(np.asarray(x, np.float32))

    pf = float(np.asarray(prefactor)); fc = float(np.asarray(factor))
    escal_np = np.zeros((128, 4), np.float32)
    escal_np[:, 2] = math.pi / 2.0
    escal_np[:, 0] = -fc * fc
    escal_np[:, 1] = math.log(max(0.5 * pf * pf, 1e-30))

    p0b2_adj = np.asarray(p0_b2, np.float64) - np.asarray(p0_w2, np.float64).sum(1)
    p1b2_adj = np.asarray(p1_b2, np.float64) - np.asarray(p1_w2, np.float64).sum(1)

    def kchunk(w):  # [S, 2k, m] -> [S, 2, 128, m]
        w = np.asarray(w, np.float32)
        return w.reshape(w.shape[0], 2, 128, w.shape[-1])

    common = {
        "w_p0w1": f32(p0_w1), "w_p0w2": f32(kchunk(p0_w2)), "w_n0": f32(n0_w),
        "w_p1w1": f32(kchunk(p1_w1)), "w_p1w2": f32(kchunk(p1_w2)),
        "w_n1": f32(n1_w), "w_f": f32(kchunk(f_w)),
        "b_p0b1": f32(np.asarray(p0_b1))[:, None, :],
        "b_p0b2": f32(p0b2_adj)[:, None, :],
        "b_n0": f32(np.asarray(n0_b))[:, None, :],
        "b_p1b1": f32(np.asarray(p1_b1))[:, None, :],
        "b_p1b2": f32(p1b2_adj)[:, None, :],
        "b_n1": f32(np.asarray(n1_b))[:, None, :],
        "b_f": f32(np.asarray(f_b))[:, None, :],
        "iota32": np.tile(np.arange(D_BLK, dtype=np.float32).astype(
            ml_dtypes.bfloat16)[None, :], (128, 1)),
        "escal": escal_np,
        "ident": np.eye(128, dtype=np.float32),
    }

    tc_np = np.asarray(total_charges, np.float32)
    layout, tot = blob_layout(APC, F_IN, H, MO, NO, c_blk)

    def pack(vals):
        blob = np.zeros(tot, np.float32)
        for name, (off, shape, dt) in layout.items():
            a = np.ascontiguousarray(np.asarray(vals[name], dt)).reshape(shape)
            raw = a.ravel().view(np.float32)
            blob[off:off + raw.size] = raw
        return blob

    in_maps = []
    for c in range(N_CORES):
        asl = slice(c * APC, (c + 1) * APC)
        spc = sp_sorted[asl]
        xT = np.ascontiguousarray(feats[perm[asl]].T)
        spid_c = np.tile(spc.astype(np.float32)[None, :], (128, 1)).astype(
            ml_dtypes.bfloat16)
        vals = {
            "xT": xT, "spid": spid_c,
            "eidx": eidx_np[c], "destid": destid_np[c], "edist": edist_np[c],
            "tc_in": tc_np[c * (B // N_CORES):(c + 1) * (B // N_CORES)][None, :],
            **common,
        }
        in_maps.append({"blob": pack(vals)})
    meta = dict(perm=perm, B=B, A=A, APC=APC, c_blk=c_blk,
                F_IN=F_IN, H=H, MO=MO, NO=NO)
    return in_maps, meta


# ---------------------------------------------------------------- runner
class SpmdRunner:
    def __init__(self, nc, n_cores=N_CORES):
        import jax
        from concourse import bass2jax
        from concourse.bass2jax import _bass_exec_p, install_neuronx_cc_hook
        from jax.sharding import Mesh, PartitionSpec
        from jax.experimental.shard_map import shard_map
        install_neuronx_cc_hook()
        self.jax = jax
        self.nc = nc
        self.n_cores = n_cores
        in_names, out_names, out_avals, zero_outs = [], [], [], []
        partition_name = (nc.partition_id_tensor.name
                          if nc.partition_id_tensor else None)
        for alloc in nc.m.functions[0].allocations:
            if not isinstance(alloc, mybir.MemoryLocationSet):
                continue
            name = alloc.memorylocations[0].name
            if alloc.kind == "ExternalInput":
                if name != partition_name:
                    in_names.append(name)
            elif alloc.kind == "ExternalOutput":
                shape = tuple(alloc.tensor_shape)
                dtype = mybir.dt.np(alloc.dtype)
                out_names.append(name)
                out_avals.append(jax.core.ShapedArray(shape, dtype))
                zero_outs.append(np.zeros(shape, dtype))
        n_params = len(in_names)
        all_in = in_names + out_names
        if partition_name is not None:
            all_in.append(partition_name)

        def _body(*args):
            operands = list(args)
            if partition_name is not None:
                operands.append(bass2jax.partition_id_tensor())
            outs = _bass_exec_p.bind(
                *operands, out_avals=tuple(out_avals), in_names=tuple(all_in),
                out_names=tuple(out_names), lowering_input_output_aliases=(),
                sim_require_finite=True, sim_require_nnan=True, nc=nc)
            return tuple(outs)

        devices = jax.devices()[:n_cores]
        mesh = Mesh(np.asarray(devices), ("core",))
        in_specs = (PartitionSpec("core"),) * (n_params + len(out_names))
        out_specs = (PartitionSpec("core"),) * len(out_names)
        self._fn = jax.jit(
            shard_map(_body, mesh=mesh, in_specs=in_specs,
                      out_specs=out_specs, check_rep=False),
            keep_unused=True)
        self.in_names, self.out_names = in_names, out_names
        self.out_avals, self.zero_outs = out_avals, zero_outs
        self.n_params = n_params

    def prepare(self, in_maps):
        per_core = [[np.asarray(m[n]) for n in self.in_names] for m in in_maps]
        concat_in = [
            np.concatenate([per_core[c][i] for c in range(self.n_cores)], axis=0)
            for i in range(self.n_params)]
        concat_zeros = [
            np.zeros((self.n_cores * z.shape[0], *z.shape[1:]), z.dtype)
            for z in self.zero_outs]
        args = [self.jax.device_put(a) for a in concat_in + concat_zeros]
        for a in args:
            a.block_until_ready()
        self._args = args

    def run(self):
        outs = self._fn(*self._args)
        self.jax.block_until_ready(outs)
        return outs

    def run_async(self):
        return self._fn(*self._args)

    def results(self, outs):
        return [
            {name: np.asarray(outs[i]).reshape(
                self.n_cores, *self.out_avals[i].shape)[c]
             for i, name in enumerate(self.out_names)}
            for c in range(self.n_cores)]


def jax_block(x):
    import jax
    jax.block_until_ready(x)


_CACHE = {}


def _get_runner(apc, f_in, h_dim, mo, no, c_blk):
    key = (apc, f_in, h_dim, mo, no, c_blk)
    if key not in _CACHE:
        nc = build_program(apc, f_in, h_dim, mo, no, c_blk)
        _CACHE[key] = SpmdRunner(nc, N_CORES)
    return _CACHE[key]


def kernel(**inputs):
    species = inputs["species"]
    in_maps, meta = prepare_inputs(**inputs)
    r = _get_runner(meta["APC"], meta["F_IN"], meta["H"], meta["MO"],
                    meta["NO"], meta["c_blk"])
    r.prepare(in_maps)
    outs = r.run()
    res = r.results(outs)
    N = meta["B"] * meta["A"]
    prech = np.empty(N, np.float32)
    charg = np.empty(N, np.float32)
    for c in range(N_CORES):
        asl = slice(c * meta["APC"], (c + 1) * meta["APC"])
        prech[meta["perm"][asl]] = res[c]["out"][0]
        charg[meta["perm"][asl]] = res[c]["out"][1]
    B, A = meta["B"], meta["A"]
    return species, charg.reshape(B, A), prech.reshape(B, A)



# revision 18
# speedup vs baseline: 8.1927x; 2.5260x over previous
"""Trainium2 Bass kernel for nn_LocalMessagePassing (2-pass GNN message passing).

8 NeuronCores, SPMD, data-parallel over molecules (4096 atoms/core):
- species routing via masked PSUM accumulation (4 species matmuls + K=1
  outer-product bias matmuls into the same accumulator)
- celu(z) = max(z,0)+exp(min(z,0))-1, the -1 folded into next-layer bias
- per pass: slice MLP -> bf16 neigh table -> AllGather -> dma_gather of edge
  contributions (dest-sorted, per-32-dest-block padded) -> segment-sum via
  matmul (mergedT += gathered^T @ (onehot*edge_w)) accumulated in PSUM
- final routed linear (M=1 matmuls) + per-molecule charge correction on chip
"""
import sys
sys.path.insert(0, "/opt/trn_rl_repo")
import math
import numpy as np
import ml_dtypes

import concourse.bacc as bacc
import concourse.mybir as mybir
import concourse.tile as tile
from concourse.alu_op_type import AluOpType

BF16 = mybir.dt.bfloat16
FP32 = mybir.dt.float32
I16 = mybir.dt.int16
AF = mybir.ActivationFunctionType
F32R = mybir.dt.float32r

N_CORES = 8
S = 4
CUTOFF = 5.2
D_BLK = 32
GSUB = 7            # 128-idx chunks per dma_gather (57 ring entries)


def split_multi_waits(nc):
    """This walrus build allows one sync-wait per instruction; hoist extras
    onto same-engine NOPs placed immediately before."""
    cnt = 0
    for fn in nc.m.functions:
        for bb in fn.blocks:
            out = []
            changed = False
            for ins in bb.instructions:
                si = ins.sync_info
                if si is not None and len(si.on_wait) > 1:
                    waits = list(si.on_wait)
                    for w in waits[:-1]:
                        cnt += 1
                        out.append(mybir.InstNoOp(
                            name=f"wsplit-{cnt}", engine=ins.engine,
                            bass_nofuse=True,
                            sync_info=mybir.SyncInfo(on_wait=[w], on_update=[]),
                        ))
                    ins.sync_info = mybir.SyncInfo(
                        on_wait=[waits[-1]], on_update=list(si.on_update))
                    changed = True
                out.append(ins)
            if changed:
                bb.instructions = out
    return cnt


# ------------------------------------------------------------- blob layout
# All per-core inputs are packed into ONE f32 dram tensor; per-operand
# overhead of the tunneled PJRT execute (~1.6 ms/operand) dominated the
# baseline runtime.
def blob_layout(apc, f_in, h_dim, mo, no, c_blk):
    nblocks = apc // D_BLK
    n_chunks = nblocks * c_blk
    ngi = (n_chunks + GSUB - 1) // GSUB
    padn_g = ngi * GSUB * 128
    secs = [
        ("xT", (f_in, apc), np.float32),
        ("spid", (128, apc), ml_dtypes.bfloat16),
        ("w_p0w1", (S, f_in, h_dim), np.float32),
        ("w_p0w2", (S, 2, 128, mo), np.float32),
        ("w_n0", (S, mo, no), np.float32),
        ("w_p1w1", (S, 2, 128, h_dim), np.float32),
        ("w_p1w2", (S, 2, 128, mo), np.float32),
        ("w_n1", (S, mo, no), np.float32),
        ("w_f", (S, 2, 128, 1), np.float32),
        ("b_p0b1", (S, 1, h_dim), np.float32),
        ("b_p0b2", (S, 1, mo), np.float32),
        ("b_n0", (S, 1, no), np.float32),
        ("b_p1b1", (S, 1, h_dim), np.float32),
        ("b_p1b2", (S, 1, mo), np.float32),
        ("b_n1", (S, 1, no), np.float32),
        ("b_f", (S, 1, 1), np.float32),
        ("eidx", (128, padn_g // 16), np.int16),
        ("destid", (128, n_chunks), ml_dtypes.bfloat16),
        ("edist", (128, n_chunks), np.float32),
        ("iota32", (128, D_BLK), ml_dtypes.bfloat16),
        ("escal", (128, 4), np.float32),
        ("tc_in", (1, apc // 128), np.float32),
        ("ident", (128, 128), np.float32),
    ]
    layout = {}
    off = 0
    for name, shape, dt in secs:
        n_elem = int(np.prod(shape))
        n_f32 = n_elem * np.dtype(dt).itemsize // 4
        layout[name] = (off, shape, dt)
        off += (n_f32 + 15) // 16 * 16
    off = (off + 127) // 128 * 128
    return layout, off


# ---------------------------------------------------------------- program
def build_program(apc, f_in, h_dim, mo, no, c_blk, *, collect=True,
                  gather=True, edge=True, mlp=True, loads=True, final=True):
    nblocks = apc // D_BLK
    n_chunks = nblocks * c_blk
    ngi = (n_chunks + GSUB - 1) // GSUB
    padn_g = ngi * GSUB * 128
    n_tab = N_CORES * apc
    CT = 512
    ncts = apc // CT
    hck = h_dim // 128

    nc = bacc.Bacc("TRN2", target_bir_lowering=False, debug=False,
                   num_devices=N_CORES)

    layout, tot = blob_layout(apc, f_in, h_dim, mo, no, c_blk)
    # [128, W] shape: a 1-D input tensor makes the runtime's per-exec input
    # handling pathologically slow (~+14 ms); 2-D is handled as a normal
    # parallel transfer.
    blob2d = nc.dram_tensor("blob", [128, tot // 128], FP32,
                            kind="ExternalInput").ap()
    blob = blob2d.rearrange("p w -> (p w)")
    _mydt = {np.float32: FP32, ml_dtypes.bfloat16: BF16, np.int16: I16}

    def din(name, shape, dt):
        off, lshape, ldt = layout[name]
        assert tuple(shape) == tuple(lshape) and _mydt[ldt] == dt
        n_elem = int(np.prod(shape))
        n_f32 = n_elem * np.dtype(ldt).itemsize // 4
        ap = blob[off:off + n_f32]
        if dt != FP32:
            ap = ap.bitcast(dt)
        dims = " ".join(f"d{i}" for i in range(len(shape)))
        return ap.rearrange(f"({dims}) -> {dims}",
                            **{f"d{i}": s for i, s in enumerate(shape)})

    xT_in = din("xT", [f_in, apc], FP32)
    spid_in = din("spid", [128, apc], BF16)
    w_p0w1 = din("w_p0w1", [S, f_in, h_dim], FP32)
    w_p0w2 = din("w_p0w2", [S, 2, 128, mo], FP32)
    w_n0 = din("w_n0", [S, mo, no], FP32)
    w_p1w1 = din("w_p1w1", [S, 2, 128, h_dim], FP32)
    w_p1w2 = din("w_p1w2", [S, 2, 128, mo], FP32)
    w_n1 = din("w_n1", [S, mo, no], FP32)
    w_f = din("w_f", [S, 2, 128, 1], FP32)
    b_p0b1 = din("b_p0b1", [S, 1, h_dim], FP32)
    b_p0b2 = din("b_p0b2", [S, 1, mo], FP32)
    b_n0 = din("b_n0", [S, 1, no], FP32)
    b_p1b1 = din("b_p1b1", [S, 1, h_dim], FP32)
    b_p1b2 = din("b_p1b2", [S, 1, mo], FP32)
    b_n1 = din("b_n1", [S, 1, no], FP32)
    b_f = din("b_f", [S, 1, 1], FP32)
    eidx = din("eidx", [128, padn_g // 16], I16)
    destid = din("destid", [128, n_chunks], BF16)
    edist = din("edist", [128, n_chunks], FP32)
    iota32 = din("iota32", [128, D_BLK], BF16)
    escal = din("escal", [128, 4], FP32)
    tc_in = din("tc_in", [1, apc // 128], FP32)
    idin = din("ident", [128, 128], FP32)

    out_t = nc.dram_tensor("out", [2, apc], FP32, kind="ExternalOutput").ap()
    prech_out = out_t[0:1, :]
    charg_out = out_t[1:2, :]

    ntab_loc = [nc.dram_tensor(f"ntl{p}", [apc, no], FP32).ap() for p in range(2)]
    ntab = [nc.dram_tensor(f"ntab{p}", [n_tab, no], FP32, addr_space="Shared").ap()
            for p in range(2)]

    with tile.TileContext(nc) as tc:
        import contextlib
        with contextlib.ExitStack() as ctx:
            persist = ctx.enter_context(tc.tile_pool(name="persist", bufs=1))
            mlp = ctx.enter_context(tc.tile_pool(name="mlp", bufs=2))
            xmp = ctx.enter_context(tc.tile_pool(name="xmp", bufs=1))
            psA = ctx.enter_context(tc.tile_pool(name="psA", bufs=2, space="PSUM"))
            psT = ctx.enter_context(tc.tile_pool(name="psT", bufs=2, space="PSUM"))
            gat = ctx.enter_context(tc.tile_pool(name="gat", bufs=3))
            mkp = ctx.enter_context(tc.tile_pool(name="mkp", bufs=4))

            def loadp(ap_in, shape, dt, name):
                t = persist.tile(shape, dt, tag=name)
                if loads:
                    nc.sync.dma_start(t[:], ap_in)
                else:
                    nc.vector.memset(t[:], 0.125)
                return t

            spid_t = loadp(spid_in[:, :], [128, apc], BF16, "spid")
            iota_t = loadp(iota32[:, :], [128, D_BLK], BF16, "iota")
            escal_t = loadp(escal[:, :], [128, 4], FP32, "escal")
            destid_t = loadp(destid[:, :], [128, n_chunks], BF16, "destid")
            edist_t = loadp(edist[:, :], [128, n_chunks], FP32, "edist")
            tc_t = loadp(tc_in[:, :], [1, apc // 128], FP32, "tc")
            ident_t = loadp(idin[:, :], [128, 128], FP32, "ident")

            # simpler: explicit loads
            def wload3(ap_src, k, s_count, m, name):
                # src AP pre-rearranged to [k, s, m]
                t = persist.tile([k, s_count * m], ap_src.dtype, tag=name)
                if loads:
                    nc.sync.dma_start(
                        t[:].rearrange("k (s m) -> k s m", s=s_count), ap_src)
                else:
                    nc.vector.memset(t[:], 0.125)
                return t

            def wload4(ap_src, k, s_count, c, m, name):
                # src AP pre-rearranged to [k, s, c, m]
                t = persist.tile([k, s_count * c * m], ap_src.dtype, tag=name)
                if loads:
                    nc.sync.dma_start(
                        t[:].rearrange("k (s c m) -> k s c m", s=s_count, c=c),
                        ap_src)
                else:
                    nc.vector.memset(t[:], 0.125)
                return t

            w1t = [wload3(w_p0w1[:, :, :].rearrange("s k m -> k s m"),
                          f_in, S, h_dim, "w1t0"),
                   wload4(w_p1w1[:, :, :, :].rearrange("s c k m -> k s c m"),
                          128, S, 2, h_dim, "w1t1")]
            w2t = [wload4(w_p0w2[:, :, :, :].rearrange("s c k m -> k s c m"),
                          128, S, 2, mo, "w2t0"),
                   wload4(w_p1w2[:, :, :, :].rearrange("s c k m -> k s c m"),
                          128, S, 2, mo, "w2t1")]
            nwt = [wload3(w_n0[:, :, :].rearrange("s k m -> k s m"),
                          mo, S, no, "nwt0"),
                   wload3(w_n1[:, :, :].rearrange("s k m -> k s m"),
                          mo, S, no, "nwt1")]
            fwt = wload4(w_f[:, :, :, :].rearrange("s c k m -> k s c m"),
                         128, S, 2, 1, "fwt")
            b1t = [wload3(b_p0b1[:, :, :].rearrange("s k m -> k s m"), 1, S, h_dim, "b1t0"),
                   wload3(b_p1b1[:, :, :].rearrange("s k m -> k s m"), 1, S, h_dim, "b1t1")]
            b2t = [wload3(b_p0b2[:, :, :].rearrange("s k m -> k s m"), 1, S, mo, "b2t0"),
                   wload3(b_p1b2[:, :, :].rearrange("s k m -> k s m"), 1, S, mo, "b2t1")]
            nbt = [wload3(b_n0[:, :, :].rearrange("s k m -> k s m"), 1, S, no, "nbt0"),
                   wload3(b_n1[:, :, :].rearrange("s k m -> k s m"), 1, S, no, "nbt1")]
            fbt = wload3(b_f[:, :, :].rearrange("s k m -> k s m"), 1, S, 1, "fbt")

            # edge weights
            wtile = persist.tile([128, n_chunks], BF16, tag="wtile")
            dec = mlp.tile([128, n_chunks], FP32, tag="dec", bufs=1)
            nc.scalar.activation(dec[:], edist_t[:], AF.Exp,
                                 bias=escal_t[:, 1:2], scale=escal_t[:, 0:1])
            cut = mlp.tile([128, n_chunks], FP32, tag="cut", bufs=1)
            nc.scalar.activation(cut[:], edist_t[:], AF.Sin,
                                 bias=escal_t[:, 2:3], scale=math.pi / CUTOFF)
            nc.vector.tensor_scalar_add(cut[:], cut[:], 1.0)
            nc.vector.tensor_tensor(wtile[:], dec[:], cut[:], AluOpType.mult)

            internT = [persist.tile([mo, apc], FP32, tag=f"internT{p}",
                                    name=f"internT{p}")
                       for p in range(2)]
            mergedT = persist.tile([no, apc], FP32, tag="mergedT")
            if not edge:
                nc.vector.memset(mergedT[:], 0.125)
            if not mlp:
                for p in range(2):
                    nc.vector.memset(internT[p][:], 0.125)
                stub = persist.tile([128, no], FP32, tag="stub")
                nc.vector.memset(stub[:], 0.125)
                for p in range(2):
                    for r0 in range(0, apc, 128):
                        nc.sync.dma_start(ntab_loc[p][r0:r0 + 128, :], stub[:])

            mskp = ctx.enter_context(tc.tile_pool(name="mskp", bufs=1))
            idxp = ctx.enter_context(tc.tile_pool(name="idxp", bufs=8))
            _cur_msk = {}

            def build_masks(ct):
                for s in range(S):
                    mt = mskp.tile([128, CT], FP32, tag=f"msk{s}", name=f"msk{s}")
                    nc.vector.tensor_scalar(
                        mt[:], spid_t[:, ct * CT:ct * CT + CT], float(s), None,
                        AluOpType.is_equal)
                    _cur_msk[s] = mt

            def msl(s, ct):
                return _cur_msk[s][:]

            def msl0(s, ct):
                return _cur_msk[s][0:1, :]

            def mlp_phase(p):
                kck = 1 if p == 0 else 2
                for ct in range(ncts):
                    asl = slice(ct * CT, ct * CT + CT)
                    build_masks(ct)
                    if p == 0:
                        xseg = mlp.tile([128, CT], FP32, tag="xseg")
                        nc.sync.dma_start(xseg[:], xT_in[:, asl])
                    # masked inputs for this column tile
                    xms = {}
                    for s in range(S):
                        for kc in range(kck):
                            xm = xmp.tile([128, CT], FP32, tag=f"xm{s}_{kc}")
                            src = (xseg if p == 0
                                   else (internT[0] if kc == 0 else mergedT))
                            nc.vector.tensor_tensor(
                                xm[:], (src[:] if p == 0 else src[:, asl]),
                                msl(s, ct), AluOpType.mult)
                            xms[s, kc] = xm
                    hts = []
                    for hc in range(hck):
                        z1 = psA.tile([128, CT], FP32, tag="zz")
                        for s in range(S):
                            nc.tensor.matmul(
                                z1[:],
                                b1t[p][0:1, s * h_dim + hc * 128:
                                       s * h_dim + hc * 128 + 128],
                                msl0(s, ct), start=(s == 0), stop=False)
                        for s in range(S):
                            for kc in range(kck):
                                if p == 0:
                                    lhsT = w1t[0][:, s * h_dim + hc * 128:
                                                  s * h_dim + hc * 128 + 128]
                                else:
                                    base = s * 2 * h_dim + kc * h_dim + hc * 128
                                    lhsT = w1t[1][:, base:base + 128]
                                nc.tensor.matmul(
                                    z1[:], lhsT,
                                    xms[s, kc][:],
                                    start=False,
                                    stop=(s == S - 1 and kc == kck - 1))
                        mn = mlp.tile([128, CT], FP32, tag="mn")
                        nc.vector.tensor_scalar_min(mn[:], z1[:], 0.0)
                        ex = mlp.tile([128, CT], FP32, tag="ex")
                        nc.scalar.activation(ex[:], mn[:], AF.Exp)
                        ht = mlp.tile([128, CT], FP32, tag="ht")
                        nc.vector.tensor_scalar_max(ht[:], z1[:], 0.0)
                        nc.vector.tensor_tensor(ht[:], ht[:], ex[:], AluOpType.add)
                        hts.append(ht)
                    z2 = psA.tile([128, CT], FP32, tag="zz")
                    for s in range(S):
                        nc.tensor.matmul(
                            z2[:], b2t[p][0:1, s * mo:s * mo + 128],
                            msl0(s, ct), start=(s == 0), stop=False)
                    for s in range(S):
                        for hc in range(hck):
                            hm = mlp.tile([128, CT], FP32, tag="hm")
                            nc.vector.tensor_tensor(
                                hm[:], hts[hc][:], msl(s, ct), AluOpType.mult)
                            base = s * 2 * mo + hc * mo
                            nc.tensor.matmul(
                                z2[:], w2t[p][:, base:base + 128],
                                hm[:],
                                start=False, stop=(s == S - 1 and hc == hck - 1))
                    nc.vector.tensor_copy(internT[p][:, asl], z2[:])
                    zn = psA.tile([128, CT], FP32, tag="zz")
                    for s in range(S):
                        nc.tensor.matmul(
                            zn[:], nbt[p][0:1, s * no:s * no + 128],
                            msl0(s, ct), start=(s == 0), stop=False)
                    for s in range(S):
                        im = mlp.tile([128, CT], FP32, tag="im")
                        nc.vector.tensor_tensor(
                            im[:], internT[p][:, asl], msl(s, ct), AluOpType.mult)
                        nc.tensor.matmul(
                            zn[:], nwt[p][:, s * no:s * no + 128],
                            im[:],
                            start=False, stop=(s == S - 1))
                    nT = mlp.tile([128, CT], FP32, tag="nT")
                    nc.vector.tensor_copy(nT[:], zn[:])
                    for q in range(CT // 128):
                        pt = psT.tile([128, 128], FP32, tag="pt")
                        nc.tensor.transpose(pt[:], nT[:, q * 128:q * 128 + 128],
                                            ident_t[:])
                        rowt = mlp.tile([128, 128], FP32, tag="rowt")
                        nc.vector.tensor_copy(rowt[:], pt[:])
                        r0 = ct * CT + q * 128
                        nc.sync.dma_start(ntab_loc[p][r0:r0 + 128, :], rowt[:])

            def edge_phase(p):
                pscols = 512
                bpp = pscols // D_BLK       # blocks per psum tile (16)
                psm = None
                for gi in range(ngi):
                    it = idxp.tile([128, GSUB * 8], I16, tag="it")
                    nc.sync.dma_start(it[:], eidx[:, gi * GSUB * 8:(gi + 1) * GSUB * 8])
                    gt = gat.tile([128, GSUB, no], FP32, tag="gt")
                    if gather:
                        nc.gpsimd.dma_gather(
                            gt[:], ntab[p][:, :], it[:],
                            num_idxs=GSUB * 128, num_idxs_reg=GSUB * 128,
                            elem_size=no)
                    else:
                        nc.vector.memset(gt[:], 0.125)
                    c0 = gi * GSUB
                    nsub = min(GSUB, n_chunks - c0)
                    if nsub <= 0:
                        continue
                    csl = slice(c0, c0 + nsub)
                    mk = mkp.tile([128, GSUB * D_BLK], BF16, tag="mk")
                    nc.vector.tensor_tensor(
                        mk[:, :nsub * D_BLK].rearrange("p (c d) -> p c d", d=D_BLK),
                        destid_t[:, csl].broadcast_to([128, nsub, D_BLK]),
                        iota_t[:].rearrange("p (x d) -> p x d", x=1)
                              .broadcast_to([128, nsub, D_BLK]),
                        AluOpType.is_equal)
                    wm = mkp.tile([128, GSUB * D_BLK], FP32, tag="wm")
                    nc.vector.tensor_tensor(
                        wm[:, :nsub * D_BLK].rearrange("p (c d) -> p c d", d=D_BLK),
                        mk[:, :nsub * D_BLK].rearrange("p (c d) -> p c d", d=D_BLK),
                        wtile[:, csl].broadcast_to([128, nsub, D_BLK]),
                        AluOpType.mult)
                    for sub in range(nsub):
                        g = c0 + sub
                        blk = g // c_blk
                        cin = g % c_blk
                        if blk % bpp == 0 and cin == 0:
                            psm = psT.tile([128, pscols], FP32, tag="psm")
                        col0 = (blk % bpp) * D_BLK
                        nc.tensor.matmul(
                            psm[:, col0:col0 + D_BLK],
                            gt[:, sub, :],
                            wm[:, sub * D_BLK:sub * D_BLK + D_BLK],
                            start=(cin == 0), stop=(cin == c_blk - 1))
                        if cin == c_blk - 1 and blk % bpp == bpp - 1:
                            grp = blk // bpp
                            nc.vector.tensor_copy(
                                mergedT[:, grp * pscols:(grp + 1) * pscols],
                                psm[:])

            def final_phase():
                prech = persist.tile([1, apc], FP32, tag="prech")
                if not final:
                    nc.vector.memset(prech[:], 0.125)
                    nc.sync.dma_start(prech_out[:, :], prech[:])
                    nc.sync.dma_start(charg_out[:, :], prech[:])
                    return
                for ct in range(ncts):
                    build_masks(ct)
                    zf = psT.tile([1, CT], FP32, tag="psm")
                    for s in range(S):
                        nc.tensor.matmul(
                            zf[:], fbt[0:1, s:s + 1], msl0(s, ct),
                            start=(s == 0), stop=False)
                    for s in range(S):
                        for kc in range(2):
                            src = internT[1] if kc == 0 else mergedT
                            xm = mlp.tile([128, CT], FP32, tag="xmf")
                            nc.vector.tensor_tensor(
                                xm[:], src[:, ct * CT:ct * CT + CT],
                                msl(s, ct), AluOpType.mult)
                            nc.tensor.matmul(
                                zf[:], fwt[:, s * 2 + kc:s * 2 + kc + 1],
                                xm[:],
                                start=False,
                                stop=(s == S - 1 and kc == 1))
                    nc.vector.tensor_copy(prech[0:1, ct * CT:ct * CT + CT], zf[:])
                nc.sync.dma_start(prech_out[:, :], prech[:])
                nmol = apc // 128
                tpre = mlp.tile([1, nmol], FP32, tag="tpre")
                nc.vector.tensor_reduce(
                    tpre[0:1, :],
                    prech[0:1, :].rearrange("p (m a) -> p m a", a=128),
                    mybir.AxisListType.X, AluOpType.add)
                corr = mlp.tile([1, nmol], FP32, tag="corr")
                nc.vector.tensor_tensor(corr[:], tc_t[:], tpre[:],
                                        AluOpType.subtract)
                nc.vector.tensor_scalar_mul(corr[:], corr[:], 1.0 / 128.0)
                nc.vector.tensor_tensor(
                    prech[0:1, :].rearrange("p (m a) -> p m a", a=128),
                    prech[0:1, :].rearrange("p (m a) -> p m a", a=128),
                    corr[0:1, :].broadcast_to([1, nmol, 128]),
                    AluOpType.add)
                nc.sync.dma_start(charg_out[:, :], prech[:])

            for p in range(2):
                if mlp:
                    mlp_phase(p)
                if collect:
                    nc.gpsimd.collective_compute(
                        "AllGather", AluOpType.bypass,
                        replica_groups=[list(range(N_CORES))],
                        ins=[ntab_loc[p]], outs=[ntab[p]])
                if edge:
                    edge_phase(p)
            final_phase()

    nc.compile()
    split_multi_waits(nc)
    return nc


# ---------------------------------------------------------------- host prep
def _wrap_idx(flat_idx):
    n = len(flat_idx)
    a = np.zeros((16, (n + 15) // 16), np.int16)
    a[np.arange(n) % 16, np.arange(n) // 16] = flat_idx
    return np.tile(a, (8, 1))


def prepare_inputs(species, in_features, atom_index12, distances, total_charges,
                   p0_w1, p0_b1, p0_w2, p0_b2, n0_w, n0_b,
                   p1_w1, p1_b1, p1_w2, p1_b2, n1_w, n1_b,
                   f_w, f_b, prefactor, factor):
    B, A = np.asarray(species).shape
    N = B * A
    F_IN = np.asarray(in_features).shape[-1]
    H = np.asarray(p0_w1).shape[-1]
    MO = np.asarray(p0_w2).shape[-1]
    NO = np.asarray(n0_w).shape[-1]
    APC = N // N_CORES
    sp = np.asarray(species).reshape(-1).astype(np.int64)
    feats = np.asarray(in_features, np.float32).reshape(N, F_IN)

    # atoms keep their natural order (core c owns [c*APC, (c+1)*APC));
    # species routing is fully mask-based so no sorting is needed, and
    # molecule boundaries (128-atom groups) stay intact for the charge
    # correction.
    perm = np.arange(N)
    inv = perm
    sp_sorted = sp

    i0 = inv[np.asarray(atom_index12[0], np.int64)]
    i1 = inv[np.asarray(atom_index12[1], np.int64)]
    dd = np.asarray(distances, np.float32)
    dest = np.concatenate([i0, i1])
    src = np.concatenate([i1, i0])
    ddist = np.concatenate([dd, dd])

    nblocks = APC // D_BLK
    dcore = dest // APC
    dloc = dest - dcore * APC
    dblk = dloc // D_BLK

    counts = np.bincount(dcore * nblocks + dblk, minlength=N_CORES * nblocks)
    c_blk = int(np.ceil(counts.max() / 128.0))
    n_chunks = nblocks * c_blk
    ngi = (n_chunks + GSUB - 1) // GSUB
    padn_g = ngi * GSUB * 128
    slots = n_chunks * 128

    key = dcore * nblocks + dblk
    order = np.argsort(key, kind="stable")
    bounds = np.searchsorted(key[order], np.arange(N_CORES * nblocks + 1))

    eidx_np = np.zeros((N_CORES, 128, padn_g // 16), np.int16)
    destid_np = np.zeros((N_CORES, 128, n_chunks), ml_dtypes.bfloat16)
    edist_np = np.zeros((N_CORES, 128, n_chunks), np.float32)
    j = np.arange(slots)
    for c in range(N_CORES):
        idx_flat = np.zeros(slots, np.int64)
        did_flat = np.full(slots, float(D_BLK), np.float32)   # pad -> no match
        dst_flat = np.zeros(slots, np.float32)
        for b in range(nblocks):
            g0, g1 = bounds[c * nblocks + b], bounds[c * nblocks + b + 1]
            cnt = g1 - g0
            s0 = b * c_blk * 128
            sel = order[g0:g1]
            idx_flat[s0:s0 + cnt] = src[sel]
            did_flat[s0:s0 + cnt] = (dloc[sel] % D_BLK).astype(np.float32)
            dst_flat[s0:s0 + cnt] = ddist[sel]
        eidx_np[c] = _wrap_idx(np.concatenate(
            [idx_flat, np.zeros(padn_g - slots, np.int64)]).astype(np.int16))
        destid_np[c, j % 128, j // 128] = did_flat.astype(ml_dtypes.bfloat16)
        edist_np[c, j % 128, j // 128] = dst_flat

    def f32(x):
        return np.ascontiguousarray(np.asarray(x, np.float32))

    pf = float(np.asarray(prefactor)); fc = float(np.asarray(factor))
    escal_np = np.zeros((128, 4), np.float32)
    escal_np[:, 2] = math.pi / 2.0
    escal_np[:, 0] = -fc * fc
    escal_np[:, 1] = math.log(max(0.5 * pf * pf, 1e-30))

    p0b2_adj = np.asarray(p0_b2, np.float64) - np.asarray(p0_w2, np.float64).sum(1)
    p1b2_adj = np.asarray(p1_b2, np.float64) - np.asarray(p1_w2, np.float64).sum(1)

    def kchunk(w):  # [S, 2k, m] -> [S, 2, 128, m]
        w = np.asarray(w, np.float32)
        return w.reshape(w.shape[0], 2, 128, w.shape[-1])

    common = {
        "w_p0w1": f32(p0_w1), "w_p0w2": f32(kchunk(p0_w2)), "w_n0": f32(n0_w),
        "w_p1w1": f32(kchunk(p1_w1)), "w_p1w2": f32(kchunk(p1_w2)),
        "w_n1": f32(n1_w), "w_f": f32(kchunk(f_w)),
        "b_p0b1": f32(np.asarray(p0_b1))[:, None, :],
        "b_p0b2": f32(p0b2_adj)[:, None, :],
        "b_n0": f32(np.asarray(n0_b))[:, None, :],
        "b_p1b1": f32(np.asarray(p1_b1))[:, None, :],
        "b_p1b2": f32(p1b2_adj)[:, None, :],
        "b_n1": f32(np.asarray(n1_b))[:, None, :],
        "b_f": f32(np.asarray(f_b))[:, None, :],
        "iota32": np.tile(np.arange(D_BLK, dtype=np.float32).astype(
            ml_dtypes.bfloat16)[None, :], (128, 1)),
        "escal": escal_np,
        "ident": np.eye(128, dtype=np.float32),
    }

    tc_np = np.asarray(total_charges, np.float32)
    layout, tot = blob_layout(APC, F_IN, H, MO, NO, c_blk)

    def pack(vals):
        blob = np.zeros(tot, np.float32)
        for name, (off, shape, dt) in layout.items():
            a = np.ascontiguousarray(np.asarray(vals[name], dt)).reshape(shape)
            raw = a.ravel().view(np.float32)
            blob[off:off + raw.size] = raw
        return blob.reshape(128, tot // 128)

    in_maps = []
    for c in range(N_CORES):
        asl = slice(c * APC, (c + 1) * APC)
        spc = sp_sorted[asl]
        xT = np.ascontiguousarray(feats[perm[asl]].T)
        spid_c = np.tile(spc.astype(np.float32)[None, :], (128, 1)).astype(
            ml_dtypes.bfloat16)
        vals = {
            "xT": xT, "spid": spid_c,
            "eidx": eidx_np[c], "destid": destid_np[c], "edist": edist_np[c],
            "tc_in": tc_np[c * (B // N_CORES):(c + 1) * (B // N_CORES)][None, :],
            **common,
        }
        in_maps.append({"blob": pack(vals)})
    meta = dict(perm=perm, B=B, A=A, APC=APC, c_blk=c_blk,
                F_IN=F_IN, H=H, MO=MO, NO=NO)
    return in_maps, meta


# ---------------------------------------------------------------- runner
class SpmdRunner:
    def __init__(self, nc, n_cores=N_CORES):
        import jax
        from concourse import bass2jax
        from concourse.bass2jax import _bass_exec_p, install_neuronx_cc_hook
        from jax.sharding import Mesh, PartitionSpec
        from jax.experimental.shard_map import shard_map
        install_neuronx_cc_hook()
        self.jax = jax
        self.nc = nc
        self.n_cores = n_cores
        in_names, out_names, out_avals, zero_outs = [], [], [], []
        partition_name = (nc.partition_id_tensor.name
                          if nc.partition_id_tensor else None)
        for alloc in nc.m.functions[0].allocations:
            if not isinstance(alloc, mybir.MemoryLocationSet):
                continue
            name = alloc.memorylocations[0].name
            if alloc.kind == "ExternalInput":
                if name != partition_name:
                    in_names.append(name)
            elif alloc.kind == "ExternalOutput":
                shape = tuple(alloc.tensor_shape)
                dtype = mybir.dt.np(alloc.dtype)
                out_names.append(name)
                out_avals.append(jax.core.ShapedArray(shape, dtype))
                zero_outs.append(np.zeros(shape, dtype))
        n_params = len(in_names)
        all_in = in_names + out_names
        if partition_name is not None:
            all_in.append(partition_name)

        def _body(*args):
            operands = list(args)
            if partition_name is not None:
                operands.append(bass2jax.partition_id_tensor())
            outs = _bass_exec_p.bind(
                *operands, out_avals=tuple(out_avals), in_names=tuple(all_in),
                out_names=tuple(out_names), lowering_input_output_aliases=(),
                sim_require_finite=True, sim_require_nnan=True, nc=nc)
            return tuple(outs)

        devices = jax.devices()[:n_cores]
        mesh = Mesh(np.asarray(devices), ("core",))
        in_specs = (PartitionSpec("core"),) * (n_params + len(out_names))
        out_specs = (PartitionSpec("core"),) * len(out_names)
        self._fn = jax.jit(
            shard_map(_body, mesh=mesh, in_specs=in_specs,
                      out_specs=out_specs, check_rep=False),
            keep_unused=True)
        self.in_names, self.out_names = in_names, out_names
        self.out_avals, self.zero_outs = out_avals, zero_outs
        self.n_params = n_params

    def prepare(self, in_maps):
        per_core = [[np.asarray(m[n]) for n in self.in_names] for m in in_maps]
        concat_in = [
            np.concatenate([per_core[c][i] for c in range(self.n_cores)], axis=0)
            for i in range(self.n_params)]
        concat_zeros = [
            np.zeros((self.n_cores * z.shape[0], *z.shape[1:]), z.dtype)
            for z in self.zero_outs]
        args = [self.jax.device_put(a) for a in concat_in + concat_zeros]
        for a in args:
            a.block_until_ready()
        self._args = args

    def run(self):
        outs = self._fn(*self._args)
        self.jax.block_until_ready(outs)
        return outs

    def run_async(self):
        return self._fn(*self._args)

    def results(self, outs):
        return [
            {name: np.asarray(outs[i]).reshape(
                self.n_cores, *self.out_avals[i].shape)[c]
             for i, name in enumerate(self.out_names)}
            for c in range(self.n_cores)]


def jax_block(x):
    import jax
    jax.block_until_ready(x)


_CACHE = {}


def _get_runner(apc, f_in, h_dim, mo, no, c_blk):
    key = (apc, f_in, h_dim, mo, no, c_blk)
    if key not in _CACHE:
        nc = build_program(apc, f_in, h_dim, mo, no, c_blk)
        _CACHE[key] = SpmdRunner(nc, N_CORES)
    return _CACHE[key]


def kernel(**inputs):
    species = inputs["species"]
    in_maps, meta = prepare_inputs(**inputs)
    r = _get_runner(meta["APC"], meta["F_IN"], meta["H"], meta["MO"],
                    meta["NO"], meta["c_blk"])
    r.prepare(in_maps)
    outs = r.run()
    res = r.results(outs)
    N = meta["B"] * meta["A"]
    prech = np.empty(N, np.float32)
    charg = np.empty(N, np.float32)
    for c in range(N_CORES):
        asl = slice(c * meta["APC"], (c + 1) * meta["APC"])
        prech[meta["perm"][asl]] = res[c]["out"][0]
        charg[meta["perm"][asl]] = res[c]["out"][1]
    B, A = meta["B"], meta["A"]
    return species, charg.reshape(B, A), prech.reshape(B, A)



# revision 31
# speedup vs baseline: 10.1004x; 1.2328x over previous
"""Trainium2 Bass kernel for nn_LocalMessagePassing (2-pass GNN message passing).

8 NeuronCores, SPMD, data-parallel over molecules (4096 atoms/core):
- species routing via masked PSUM accumulation (4 species matmuls + K=1
  outer-product bias matmuls into the same accumulator)
- celu(z) = max(z,0)+exp(min(z,0))-1, the -1 folded into next-layer bias
- per pass: slice MLP -> bf16 neigh table -> AllGather -> dma_gather of edge
  contributions (dest-sorted, per-32-dest-block padded) -> segment-sum via
  matmul (mergedT += gathered^T @ (onehot*edge_w)) accumulated in PSUM
- final routed linear (M=1 matmuls) + per-molecule charge correction on chip
"""
import sys
sys.path.insert(0, "/opt/trn_rl_repo")
import math
import numpy as np
import ml_dtypes

import concourse.bacc as bacc
import concourse.mybir as mybir
import concourse.tile as tile
from concourse.alu_op_type import AluOpType

BF16 = mybir.dt.bfloat16
FP32 = mybir.dt.float32
I16 = mybir.dt.int16
AF = mybir.ActivationFunctionType
F32R = mybir.dt.float32r

N_CORES = 8
S = 4
CUTOFF = 5.2
D_BLK = 32
GSUB = 7            # 128-idx chunks per dma_gather (57 ring entries)


def split_multi_waits(nc):
    """This walrus build allows one sync-wait per instruction; hoist extras
    onto same-engine NOPs placed immediately before."""
    cnt = 0
    for fn in nc.m.functions:
        for bb in fn.blocks:
            out = []
            changed = False
            for ins in bb.instructions:
                si = ins.sync_info
                if si is not None and len(si.on_wait) > 1:
                    waits = list(si.on_wait)
                    for w in waits[:-1]:
                        cnt += 1
                        out.append(mybir.InstNoOp(
                            name=f"wsplit-{cnt}", engine=ins.engine,
                            bass_nofuse=True,
                            sync_info=mybir.SyncInfo(on_wait=[w], on_update=[]),
                        ))
                    ins.sync_info = mybir.SyncInfo(
                        on_wait=[waits[-1]], on_update=list(si.on_update))
                    changed = True
                out.append(ins)
            if changed:
                bb.instructions = out
    return cnt


# ------------------------------------------------------------- blob layout
# All per-core inputs are packed into ONE f32 dram tensor; per-operand
# overhead of the tunneled PJRT execute (~1.6 ms/operand) dominated the
# baseline runtime.
def blob_layout(apc, f_in, h_dim, mo, no, c_blk):
    nblocks = apc // D_BLK
    n_chunks = nblocks * c_blk
    ngi = (n_chunks + GSUB - 1) // GSUB
    padn_g = ngi * GSUB * 128
    secs = [
        ("xT", (f_in, apc), np.float32),
        ("spid", (128, apc), ml_dtypes.bfloat16),
        ("w_p0w1", (S, f_in, h_dim), np.float32),
        ("w_p0w2", (S, 2, 128, mo), np.float32),
        ("w_n0", (S, mo, no), np.float32),
        ("w_p1w1", (S, 2, 128, h_dim), np.float32),
        ("w_p1w2", (S, 2, 128, mo), np.float32),
        ("w_n1", (S, mo, no), np.float32),
        ("w_f", (S, 2, 128, 1), np.float32),
        ("b_p0b1", (S, 1, h_dim), np.float32),
        ("b_p0b2", (S, 1, mo), np.float32),
        ("b_n0", (S, 1, no), np.float32),
        ("b_p1b1", (S, 1, h_dim), np.float32),
        ("b_p1b2", (S, 1, mo), np.float32),
        ("b_n1", (S, 1, no), np.float32),
        ("b_f", (S, 1, 1), np.float32),
        ("eidx", (16, padn_g // 16), np.int16),
        ("destid", (128, n_chunks), ml_dtypes.bfloat16),
        ("wtile", (128, n_chunks), ml_dtypes.bfloat16),
        ("iota32", (128, D_BLK), ml_dtypes.bfloat16),
        ("tc_in", (1, apc // 128), np.float32),
        ("ident", (128, 128), np.float32),
    ]
    layout = {}
    off = 0
    for name, shape, dt in secs:
        n_elem = int(np.prod(shape))
        n_f32 = n_elem * np.dtype(dt).itemsize // 4
        layout[name] = (off, shape, dt)
        off += (n_f32 + 15) // 16 * 16
    off = (off + 127) // 128 * 128
    return layout, off


# ---------------------------------------------------------------- program
def build_program(apc, f_in, h_dim, mo, no, c_blk, *, collect=True,
                  gather=True, edge=True, mlp=True, loads=True, final=True):
    nblocks = apc // D_BLK
    n_chunks = nblocks * c_blk
    ngi = (n_chunks + GSUB - 1) // GSUB
    padn_g = ngi * GSUB * 128
    n_tab = N_CORES * apc
    CT = 512
    ncts = apc // CT
    hck = h_dim // 128

    nc = bacc.Bacc("TRN2", target_bir_lowering=False, debug=False,
                   num_devices=N_CORES)

    layout, tot = blob_layout(apc, f_in, h_dim, mo, no, c_blk)
    # [128, W] shape: a 1-D input tensor makes the runtime's per-exec input
    # handling pathologically slow (~+14 ms); 2-D is handled as a normal
    # parallel transfer.
    blob2d = nc.dram_tensor("blob", [128, tot // 128], FP32,
                            kind="ExternalInput").ap()
    blob = blob2d.rearrange("p w -> (p w)")
    _mydt = {np.float32: FP32, ml_dtypes.bfloat16: BF16, np.int16: I16}

    def din(name, shape, dt):
        off, lshape, ldt = layout[name]
        assert tuple(shape) == tuple(lshape) and _mydt[ldt] == dt
        n_elem = int(np.prod(shape))
        n_f32 = n_elem * np.dtype(ldt).itemsize // 4
        ap = blob[off:off + n_f32]
        if dt != FP32:
            ap = ap.bitcast(dt)
        dims = " ".join(f"d{i}" for i in range(len(shape)))
        return ap.rearrange(f"({dims}) -> {dims}",
                            **{f"d{i}": s for i, s in enumerate(shape)})

    xT_in = din("xT", [f_in, apc], FP32)
    spid_in = din("spid", [128, apc], BF16)
    w_p0w1 = din("w_p0w1", [S, f_in, h_dim], FP32)
    w_p0w2 = din("w_p0w2", [S, 2, 128, mo], FP32)
    w_n0 = din("w_n0", [S, mo, no], FP32)
    w_p1w1 = din("w_p1w1", [S, 2, 128, h_dim], FP32)
    w_p1w2 = din("w_p1w2", [S, 2, 128, mo], FP32)
    w_n1 = din("w_n1", [S, mo, no], FP32)
    w_f = din("w_f", [S, 2, 128, 1], FP32)
    b_p0b1 = din("b_p0b1", [S, 1, h_dim], FP32)
    b_p0b2 = din("b_p0b2", [S, 1, mo], FP32)
    b_n0 = din("b_n0", [S, 1, no], FP32)
    b_p1b1 = din("b_p1b1", [S, 1, h_dim], FP32)
    b_p1b2 = din("b_p1b2", [S, 1, mo], FP32)
    b_n1 = din("b_n1", [S, 1, no], FP32)
    b_f = din("b_f", [S, 1, 1], FP32)
    eidx = din("eidx", [16, padn_g // 16], I16)
    destid = din("destid", [128, n_chunks], BF16)
    wtile_in = din("wtile", [128, n_chunks], BF16)
    iota32 = din("iota32", [128, D_BLK], BF16)
    tc_in = din("tc_in", [1, apc // 128], FP32)
    idin = din("ident", [128, 128], FP32)

    out_t = nc.dram_tensor("out", [2, apc], FP32, kind="ExternalOutput").ap()
    prech_out = out_t[0:1, :]
    charg_out = out_t[1:2, :]

    ntab_loc = [nc.dram_tensor(f"ntl{p}", [apc, no], BF16).ap() for p in range(2)]
    ntab = [nc.dram_tensor(f"ntab{p}", [n_tab, no], BF16, addr_space="Shared").ap()
            for p in range(2)]

    with tile.TileContext(nc) as tc:
        import contextlib
        with contextlib.ExitStack() as ctx:
            persist = ctx.enter_context(tc.tile_pool(name="persist", bufs=1))
            mlp = ctx.enter_context(tc.tile_pool(name="mlp", bufs=2))
            xmp = ctx.enter_context(tc.tile_pool(name="xmp", bufs=1))
            psA = ctx.enter_context(tc.tile_pool(name="psA", bufs=2, space="PSUM"))
            psT = ctx.enter_context(tc.tile_pool(name="psT", bufs=2, space="PSUM"))
            gat = ctx.enter_context(tc.tile_pool(name="gat", bufs=3))
            mkp = ctx.enter_context(tc.tile_pool(name="mkp", bufs=4))

            def loadp(ap_in, shape, dt, name):
                t = persist.tile(shape, dt, tag=name)
                if loads:
                    nc.sync.dma_start(t[:], ap_in)
                else:
                    nc.vector.memset(t[:], 0.125)
                return t

            spid_t = loadp(spid_in[:, :], [128, apc], BF16, "spid")
            iota_t = loadp(iota32[:, :], [128, D_BLK], BF16, "iota")
            destid_t = loadp(destid[:, :], [128, n_chunks], BF16, "destid")
            wtile = loadp(wtile_in[:, :], [128, n_chunks], BF16, "wtile")
            tc_t = loadp(tc_in[:, :], [1, apc // 128], FP32, "tc")
            ident_t = loadp(idin[:, :], [128, 128], FP32, "ident")

            # edge-gather indices: stored once as [16, X] in the blob,
            # replicated to 128 partitions on chip (log-doubling SBUF DMAs)
            eidx_t = persist.tile([128, padn_g // 16], I16, tag="eidxf")
            if loads:
                nc.sync.dma_start(eidx_t[0:16, :], eidx)
                for k in (16, 32, 64):
                    nc.sync.dma_start(eidx_t[k:2 * k, :], eidx_t[0:k, :])
            else:
                nc.vector.memset(eidx_t[:], 0)

            # simpler: explicit loads
            def wload3(ap_src, k, s_count, m, name):
                # src AP pre-rearranged to [k, s, m]
                t = persist.tile([k, s_count * m], ap_src.dtype, tag=name)
                if loads:
                    nc.sync.dma_start(
                        t[:].rearrange("k (s m) -> k s m", s=s_count), ap_src)
                else:
                    nc.vector.memset(t[:], 0.125)
                return t

            def wload4(ap_src, k, s_count, c, m, name):
                # src AP pre-rearranged to [k, s, c, m]
                t = persist.tile([k, s_count * c * m], ap_src.dtype, tag=name)
                if loads:
                    nc.sync.dma_start(
                        t[:].rearrange("k (s c m) -> k s c m", s=s_count, c=c),
                        ap_src)
                else:
                    nc.vector.memset(t[:], 0.125)
                return t

            w1t = [wload3(w_p0w1[:, :, :].rearrange("s k m -> k s m"),
                          f_in, S, h_dim, "w1t0"),
                   wload4(w_p1w1[:, :, :, :].rearrange("s c k m -> k s c m"),
                          128, S, 2, h_dim, "w1t1")]
            w2t = [wload4(w_p0w2[:, :, :, :].rearrange("s c k m -> k s c m"),
                          128, S, 2, mo, "w2t0"),
                   wload4(w_p1w2[:, :, :, :].rearrange("s c k m -> k s c m"),
                          128, S, 2, mo, "w2t1")]
            nwt = [wload3(w_n0[:, :, :].rearrange("s k m -> k s m"),
                          mo, S, no, "nwt0"),
                   wload3(w_n1[:, :, :].rearrange("s k m -> k s m"),
                          mo, S, no, "nwt1")]
            fwt = wload4(w_f[:, :, :, :].rearrange("s c k m -> k s c m"),
                         128, S, 2, 1, "fwt")
            b1t = [wload3(b_p0b1[:, :, :].rearrange("s k m -> k s m"), 1, S, h_dim, "b1t0"),
                   wload3(b_p1b1[:, :, :].rearrange("s k m -> k s m"), 1, S, h_dim, "b1t1")]
            b2t = [wload3(b_p0b2[:, :, :].rearrange("s k m -> k s m"), 1, S, mo, "b2t0"),
                   wload3(b_p1b2[:, :, :].rearrange("s k m -> k s m"), 1, S, mo, "b2t1")]
            nbt = [wload3(b_n0[:, :, :].rearrange("s k m -> k s m"), 1, S, no, "nbt0"),
                   wload3(b_n1[:, :, :].rearrange("s k m -> k s m"), 1, S, no, "nbt1")]
            fbt = wload3(b_f[:, :, :].rearrange("s k m -> k s m"), 1, S, 1, "fbt")

            internT = [persist.tile([mo, apc], FP32, tag=f"internT{p}",
                                    name=f"internT{p}")
                       for p in range(2)]
            mergedT = persist.tile([no, apc], FP32, tag="mergedT")
            if not edge:
                nc.vector.memset(mergedT[:], 0.125)
            if not mlp:
                for p in range(2):
                    nc.vector.memset(internT[p][:], 0.125)
                stub = persist.tile([128, no], FP32, tag="stub")
                nc.vector.memset(stub[:], 0.125)
                for p in range(2):
                    for r0 in range(0, apc, 128):
                        nc.sync.dma_start(ntab_loc[p][r0:r0 + 128, :], stub[:])

            mskp = ctx.enter_context(tc.tile_pool(name="mskp", bufs=1))
            _cur_msk = {}

            def build_masks(ct):
                for s in range(S):
                    mt = mskp.tile([128, CT], FP32, tag=f"msk{s}", name=f"msk{s}")
                    nc.vector.tensor_scalar(
                        mt[:], spid_t[:, ct * CT:ct * CT + CT], float(s), None,
                        AluOpType.is_equal)
                    _cur_msk[s] = mt

            def msl(s, ct):
                return _cur_msk[s][:]

            def msl0(s, ct):
                return _cur_msk[s][0:1, :]

            def mlp_phase(p):
                kck = 1 if p == 0 else 2
                for ct in range(ncts):
                    asl = slice(ct * CT, ct * CT + CT)
                    build_masks(ct)
                    if p == 0:
                        xseg = mlp.tile([128, CT], FP32, tag="xseg")
                        nc.sync.dma_start(xseg[:], xT_in[:, asl])
                    # masked inputs for this column tile
                    xms = {}
                    for s in range(S):
                        for kc in range(kck):
                            xm = xmp.tile([128, CT], FP32, tag=f"xm{s}_{kc}")
                            src = (xseg if p == 0
                                   else (internT[0] if kc == 0 else mergedT))
                            nc.vector.tensor_tensor(
                                xm[:], (src[:] if p == 0 else src[:, asl]),
                                msl(s, ct), AluOpType.mult)
                            xms[s, kc] = xm
                    hts = []
                    for hc in range(hck):
                        z1 = psA.tile([128, CT], FP32, tag="zz")
                        for s in range(S):
                            nc.tensor.matmul(
                                z1[:],
                                b1t[p][0:1, s * h_dim + hc * 128:
                                       s * h_dim + hc * 128 + 128],
                                msl0(s, ct), start=(s == 0), stop=False)
                        for s in range(S):
                            for kc in range(kck):
                                if p == 0:
                                    lhsT = w1t[0][:, s * h_dim + hc * 128:
                                                  s * h_dim + hc * 128 + 128]
                                else:
                                    base = s * 2 * h_dim + kc * h_dim + hc * 128
                                    lhsT = w1t[1][:, base:base + 128]
                                nc.tensor.matmul(
                                    z1[:], lhsT,
                                    xms[s, kc][:],
                                    start=False,
                                    stop=(s == S - 1 and kc == kck - 1))
                        mn = mlp.tile([128, CT], FP32, tag="mn")
                        nc.vector.tensor_scalar_min(mn[:], z1[:], 0.0)
                        ex = mlp.tile([128, CT], FP32, tag="ex")
                        nc.scalar.activation(ex[:], mn[:], AF.Exp)
                        ht = mlp.tile([128, CT], FP32, tag="ht")
                        nc.vector.tensor_scalar_max(ht[:], z1[:], 0.0)
                        nc.vector.tensor_tensor(ht[:], ht[:], ex[:], AluOpType.add)
                        hts.append(ht)
                    z2 = psA.tile([128, CT], FP32, tag="zz")
                    for s in range(S):
                        nc.tensor.matmul(
                            z2[:], b2t[p][0:1, s * mo:s * mo + 128],
                            msl0(s, ct), start=(s == 0), stop=False)
                    for s in range(S):
                        for hc in range(hck):
                            hm = mlp.tile([128, CT], FP32, tag="hm")
                            nc.vector.tensor_tensor(
                                hm[:], hts[hc][:], msl(s, ct), AluOpType.mult)
                            base = s * 2 * mo + hc * mo
                            nc.tensor.matmul(
                                z2[:], w2t[p][:, base:base + 128],
                                hm[:],
                                start=False, stop=(s == S - 1 and hc == hck - 1))
                    nc.vector.tensor_copy(internT[p][:, asl], z2[:])
                    zn = psA.tile([128, CT], FP32, tag="zz")
                    for s in range(S):
                        nc.tensor.matmul(
                            zn[:], nbt[p][0:1, s * no:s * no + 128],
                            msl0(s, ct), start=(s == 0), stop=False)
                    for s in range(S):
                        im = mlp.tile([128, CT], FP32, tag="im")
                        nc.vector.tensor_tensor(
                            im[:], internT[p][:, asl], msl(s, ct), AluOpType.mult)
                        nc.tensor.matmul(
                            zn[:], nwt[p][:, s * no:s * no + 128],
                            im[:],
                            start=False, stop=(s == S - 1))
                    nT = mlp.tile([128, CT], FP32, tag="nT")
                    nc.vector.tensor_copy(nT[:], zn[:])
                    for q in range(CT // 128):
                        pt = psT.tile([128, 128], FP32, tag="pt")
                        nc.tensor.transpose(pt[:], nT[:, q * 128:q * 128 + 128],
                                            ident_t[:])
                        rowt = mlp.tile([128, 128], BF16, tag="rowt")
                        nc.vector.tensor_copy(rowt[:], pt[:])
                        r0 = ct * CT + q * 128
                        nc.sync.dma_start(ntab_loc[p][r0:r0 + 128, :], rowt[:])

            def edge_phase(p):
                pscols = 512
                bpp = pscols // D_BLK       # blocks per psum tile (16)
                psm = None
                for gi in range(ngi):
                    gt = gat.tile([128, GSUB, no], BF16, tag="gt")
                    if gather:
                        nc.gpsimd.dma_gather(
                            gt[:], ntab[p][:, :],
                            eidx_t[:, gi * GSUB * 8:(gi + 1) * GSUB * 8],
                            num_idxs=GSUB * 128, num_idxs_reg=GSUB * 128,
                            elem_size=no)
                    else:
                        nc.vector.memset(gt[:], 0.125)
                    c0 = gi * GSUB
                    nsub = min(GSUB, n_chunks - c0)
                    if nsub <= 0:
                        continue
                    csl = slice(c0, c0 + nsub)
                    mk = mkp.tile([128, GSUB * D_BLK], BF16, tag="mk")
                    nc.vector.tensor_tensor(
                        mk[:, :nsub * D_BLK].rearrange("p (c d) -> p c d", d=D_BLK),
                        destid_t[:, csl].broadcast_to([128, nsub, D_BLK]),
                        iota_t[:].rearrange("p (x d) -> p x d", x=1)
                              .broadcast_to([128, nsub, D_BLK]),
                        AluOpType.is_equal)
                    wm = mkp.tile([128, GSUB * D_BLK], BF16, tag="wm")
                    nc.vector.tensor_tensor(
                        wm[:, :nsub * D_BLK].rearrange("p (c d) -> p c d", d=D_BLK),
                        mk[:, :nsub * D_BLK].rearrange("p (c d) -> p c d", d=D_BLK),
                        wtile[:, csl].broadcast_to([128, nsub, D_BLK]),
                        AluOpType.mult)
                    for sub in range(nsub):
                        g = c0 + sub
                        blk = g // c_blk
                        cin = g % c_blk
                        if blk % bpp == 0 and cin == 0:
                            psm = psT.tile([128, pscols], FP32, tag="psm")
                        col0 = (blk % bpp) * D_BLK
                        nc.tensor.matmul(
                            psm[:, col0:col0 + D_BLK],
                            gt[:, sub, :],
                            wm[:, sub * D_BLK:sub * D_BLK + D_BLK],
                            start=(cin == 0), stop=(cin == c_blk - 1))
                        if cin == c_blk - 1 and blk % bpp == bpp - 1:
                            grp = blk // bpp
                            nc.vector.tensor_copy(
                                mergedT[:, grp * pscols:(grp + 1) * pscols],
                                psm[:])

            def final_phase():
                prech = persist.tile([1, apc], FP32, tag="prech")
                if not final:
                    nc.vector.memset(prech[:], 0.125)
                    nc.sync.dma_start(prech_out[:, :], prech[:])
                    nc.sync.dma_start(charg_out[:, :], prech[:])
                    return
                for ct in range(ncts):
                    build_masks(ct)
                    zf = psT.tile([1, CT], FP32, tag="psm")
                    for s in range(S):
                        nc.tensor.matmul(
                            zf[:], fbt[0:1, s:s + 1], msl0(s, ct),
                            start=(s == 0), stop=False)
                    for s in range(S):
                        for kc in range(2):
                            src = internT[1] if kc == 0 else mergedT
                            xm = mlp.tile([128, CT], FP32, tag="xmf")
                            nc.vector.tensor_tensor(
                                xm[:], src[:, ct * CT:ct * CT + CT],
                                msl(s, ct), AluOpType.mult)
                            nc.tensor.matmul(
                                zf[:], fwt[:, s * 2 + kc:s * 2 + kc + 1],
                                xm[:],
                                start=False,
                                stop=(s == S - 1 and kc == 1))
                    nc.vector.tensor_copy(prech[0:1, ct * CT:ct * CT + CT], zf[:])
                nc.sync.dma_start(prech_out[:, :], prech[:])
                nmol = apc // 128
                tpre = mlp.tile([1, nmol], FP32, tag="tpre")
                nc.vector.tensor_reduce(
                    tpre[0:1, :],
                    prech[0:1, :].rearrange("p (m a) -> p m a", a=128),
                    mybir.AxisListType.X, AluOpType.add)
                corr = mlp.tile([1, nmol], FP32, tag="corr")
                nc.vector.tensor_tensor(corr[:], tc_t[:], tpre[:],
                                        AluOpType.subtract)
                nc.vector.tensor_scalar_mul(corr[:], corr[:], 1.0 / 128.0)
                nc.vector.tensor_tensor(
                    prech[0:1, :].rearrange("p (m a) -> p m a", a=128),
                    prech[0:1, :].rearrange("p (m a) -> p m a", a=128),
                    corr[0:1, :].broadcast_to([1, nmol, 128]),
                    AluOpType.add)
                nc.sync.dma_start(charg_out[:, :], prech[:])

            for p in range(2):
                if mlp:
                    mlp_phase(p)
                if collect:
                    nc.gpsimd.collective_compute(
                        "AllGather", AluOpType.bypass,
                        replica_groups=[list(range(N_CORES))],
                        ins=[ntab_loc[p]], outs=[ntab[p]])
                if edge:
                    edge_phase(p)
            final_phase()

    nc.compile()
    split_multi_waits(nc)
    return nc


# ---------------------------------------------------------------- host prep
def _wrap_idx(flat_idx):
    n = len(flat_idx)
    a = np.zeros((16, (n + 15) // 16), np.int16)
    a[np.arange(n) % 16, np.arange(n) // 16] = flat_idx
    return a


def prepare_inputs(species, in_features, atom_index12, distances, total_charges,
                   p0_w1, p0_b1, p0_w2, p0_b2, n0_w, n0_b,
                   p1_w1, p1_b1, p1_w2, p1_b2, n1_w, n1_b,
                   f_w, f_b, prefactor, factor):
    B, A = np.asarray(species).shape
    N = B * A
    F_IN = np.asarray(in_features).shape[-1]
    H = np.asarray(p0_w1).shape[-1]
    MO = np.asarray(p0_w2).shape[-1]
    NO = np.asarray(n0_w).shape[-1]
    APC = N // N_CORES
    sp = np.asarray(species).reshape(-1).astype(np.int64)
    feats = np.asarray(in_features, np.float32).reshape(N, F_IN)

    # atoms keep their natural order (core c owns [c*APC, (c+1)*APC));
    # species routing is fully mask-based so no sorting is needed, and
    # molecule boundaries (128-atom groups) stay intact for the charge
    # correction.
    perm = np.arange(N)
    inv = perm
    sp_sorted = sp

    i0 = inv[np.asarray(atom_index12[0], np.int64)]
    i1 = inv[np.asarray(atom_index12[1], np.int64)]
    dd = np.asarray(distances, np.float32)
    dest = np.concatenate([i0, i1])
    src = np.concatenate([i1, i0])
    ddist = np.concatenate([dd, dd])

    nblocks = APC // D_BLK
    dcore = dest // APC
    dloc = dest - dcore * APC
    dblk = dloc // D_BLK

    counts = np.bincount(dcore * nblocks + dblk, minlength=N_CORES * nblocks)
    c_blk = int(np.ceil(counts.max() / 128.0))
    n_chunks = nblocks * c_blk
    ngi = (n_chunks + GSUB - 1) // GSUB
    padn_g = ngi * GSUB * 128
    slots = n_chunks * 128

    key = dcore * nblocks + dblk
    order = np.argsort(key, kind="stable")
    bounds = np.searchsorted(key[order], np.arange(N_CORES * nblocks + 1))

    # host-precomputed edge weight (decay * cosine cutoff), fp64
    pf = float(np.asarray(prefactor)); fc = float(np.asarray(factor))
    dd64 = ddist.astype(np.float64)
    ew = (pf * pf) * np.exp(-(fc * fc) * dd64) * np.where(
        dd64 < CUTOFF, 0.5 * np.cos(np.pi * dd64 / CUTOFF) + 0.5, 0.0)

    eidx_np = np.zeros((N_CORES, 16, padn_g // 16), np.int16)
    destid_np = np.zeros((N_CORES, 128, n_chunks), ml_dtypes.bfloat16)
    wtile_np = np.zeros((N_CORES, 128, n_chunks), ml_dtypes.bfloat16)
    j = np.arange(slots)
    for c in range(N_CORES):
        idx_flat = np.zeros(slots, np.int64)
        did_flat = np.full(slots, float(D_BLK), np.float32)   # pad -> no match
        w_flat = np.zeros(slots, np.float64)
        for b in range(nblocks):
            g0, g1 = bounds[c * nblocks + b], bounds[c * nblocks + b + 1]
            cnt = g1 - g0
            s0 = b * c_blk * 128
            sel = order[g0:g1]
            sel = sel[np.argsort(src[sel], kind="stable")]  # gather locality
            idx_flat[s0:s0 + cnt] = src[sel]
            did_flat[s0:s0 + cnt] = (dloc[sel] % D_BLK).astype(np.float32)
            w_flat[s0:s0 + cnt] = ew[sel]
        eidx_np[c] = _wrap_idx(np.concatenate(
            [idx_flat, np.zeros(padn_g - slots, np.int64)]).astype(np.int16))
        destid_np[c, j % 128, j // 128] = did_flat.astype(ml_dtypes.bfloat16)
        wtile_np[c, j % 128, j // 128] = w_flat.astype(ml_dtypes.bfloat16)

    def f32(x):
        return np.ascontiguousarray(np.asarray(x, np.float32))

    p0b2_adj = np.asarray(p0_b2, np.float64) - np.asarray(p0_w2, np.float64).sum(1)
    p1b2_adj = np.asarray(p1_b2, np.float64) - np.asarray(p1_w2, np.float64).sum(1)

    def kchunk(w):  # [S, 2k, m] -> [S, 2, 128, m]
        w = np.asarray(w, np.float32)
        return w.reshape(w.shape[0], 2, 128, w.shape[-1])

    common = {
        "w_p0w1": f32(p0_w1), "w_p0w2": f32(kchunk(p0_w2)), "w_n0": f32(n0_w),
        "w_p1w1": f32(kchunk(p1_w1)), "w_p1w2": f32(kchunk(p1_w2)),
        "w_n1": f32(n1_w), "w_f": f32(kchunk(f_w)),
        "b_p0b1": f32(np.asarray(p0_b1))[:, None, :],
        "b_p0b2": f32(p0b2_adj)[:, None, :],
        "b_n0": f32(np.asarray(n0_b))[:, None, :],
        "b_p1b1": f32(np.asarray(p1_b1))[:, None, :],
        "b_p1b2": f32(p1b2_adj)[:, None, :],
        "b_n1": f32(np.asarray(n1_b))[:, None, :],
        "b_f": f32(np.asarray(f_b))[:, None, :],
        "iota32": np.tile(np.arange(D_BLK, dtype=np.float32).astype(
            ml_dtypes.bfloat16)[None, :], (128, 1)),
        "ident": np.eye(128, dtype=np.float32),
    }

    tc_np = np.asarray(total_charges, np.float32)
    layout, tot = blob_layout(APC, F_IN, H, MO, NO, c_blk)

    def pack(vals):
        blob = np.zeros(tot, np.float32)
        for name, (off, shape, dt) in layout.items():
            a = np.ascontiguousarray(np.asarray(vals[name], dt)).reshape(shape)
            raw = a.ravel().view(np.float32)
            blob[off:off + raw.size] = raw
        return blob.reshape(128, tot // 128)

    in_maps = []
    for c in range(N_CORES):
        asl = slice(c * APC, (c + 1) * APC)
        spc = sp_sorted[asl]
        xT = np.ascontiguousarray(feats[perm[asl]].T)
        spid_c = np.tile(spc.astype(np.float32)[None, :], (128, 1)).astype(
            ml_dtypes.bfloat16)
        vals = {
            "xT": xT, "spid": spid_c,
            "eidx": eidx_np[c], "destid": destid_np[c], "wtile": wtile_np[c],
            "tc_in": tc_np[c * (B // N_CORES):(c + 1) * (B // N_CORES)][None, :],
            **common,
        }
        in_maps.append({"blob": pack(vals)})
    meta = dict(perm=perm, B=B, A=A, APC=APC, c_blk=c_blk,
                F_IN=F_IN, H=H, MO=MO, NO=NO)
    return in_maps, meta


# ---------------------------------------------------------------- runner
class SpmdRunner:
    def __init__(self, nc, n_cores=N_CORES):
        import jax
        from concourse import bass2jax
        from concourse.bass2jax import _bass_exec_p, install_neuronx_cc_hook
        from jax.sharding import Mesh, PartitionSpec
        from jax.experimental.shard_map import shard_map
        install_neuronx_cc_hook()
        self.jax = jax
        self.nc = nc
        self.n_cores = n_cores
        in_names, out_names, out_avals, zero_outs = [], [], [], []
        partition_name = (nc.partition_id_tensor.name
                          if nc.partition_id_tensor else None)
        for alloc in nc.m.functions[0].allocations:
            if not isinstance(alloc, mybir.MemoryLocationSet):
                continue
            name = alloc.memorylocations[0].name
            if alloc.kind == "ExternalInput":
                if name != partition_name:
                    in_names.append(name)
            elif alloc.kind == "ExternalOutput":
                shape = tuple(alloc.tensor_shape)
                dtype = mybir.dt.np(alloc.dtype)
                out_names.append(name)
                out_avals.append(jax.core.ShapedArray(shape, dtype))
                zero_outs.append(np.zeros(shape, dtype))
        n_params = len(in_names)
        all_in = in_names + out_names
        if partition_name is not None:
            all_in.append(partition_name)

        def _body(*args):
            operands = list(args)
            if partition_name is not None:
                operands.append(bass2jax.partition_id_tensor())
            outs = _bass_exec_p.bind(
                *operands, out_avals=tuple(out_avals), in_names=tuple(all_in),
                out_names=tuple(out_names), lowering_input_output_aliases=(),
                sim_require_finite=True, sim_require_nnan=True, nc=nc)
            return tuple(outs)

        devices = jax.devices()[:n_cores]
        mesh = Mesh(np.asarray(devices), ("core",))
        in_specs = (PartitionSpec("core"),) * (n_params + len(out_names))
        out_specs = (PartitionSpec("core"),) * len(out_names)
        self._fn = jax.jit(
            shard_map(_body, mesh=mesh, in_specs=in_specs,
                      out_specs=out_specs, check_rep=False),
            keep_unused=True)
        self.in_names, self.out_names = in_names, out_names
        self.out_avals, self.zero_outs = out_avals, zero_outs
        self.n_params = n_params

    def prepare(self, in_maps):
        per_core = [[np.asarray(m[n]) for n in self.in_names] for m in in_maps]
        concat_in = [
            np.concatenate([per_core[c][i] for c in range(self.n_cores)], axis=0)
            for i in range(self.n_params)]
        concat_zeros = [
            np.zeros((self.n_cores * z.shape[0], *z.shape[1:]), z.dtype)
            for z in self.zero_outs]
        args = [self.jax.device_put(a) for a in concat_in + concat_zeros]
        for a in args:
            a.block_until_ready()
        self._args = args

    def run(self):
        outs = self._fn(*self._args)
        self.jax.block_until_ready(outs)
        return outs

    def run_async(self):
        return self._fn(*self._args)

    def results(self, outs):
        return [
            {name: np.asarray(outs[i]).reshape(
                self.n_cores, *self.out_avals[i].shape)[c]
             for i, name in enumerate(self.out_names)}
            for c in range(self.n_cores)]


def jax_block(x):
    import jax
    jax.block_until_ready(x)


_CACHE = {}


def _get_runner(apc, f_in, h_dim, mo, no, c_blk):
    key = (apc, f_in, h_dim, mo, no, c_blk)
    if key not in _CACHE:
        nc = build_program(apc, f_in, h_dim, mo, no, c_blk)
        _CACHE[key] = SpmdRunner(nc, N_CORES)
    return _CACHE[key]


def kernel(**inputs):
    species = inputs["species"]
    in_maps, meta = prepare_inputs(**inputs)
    r = _get_runner(meta["APC"], meta["F_IN"], meta["H"], meta["MO"],
                    meta["NO"], meta["c_blk"])
    r.prepare(in_maps)
    outs = r.run()
    res = r.results(outs)
    N = meta["B"] * meta["A"]
    prech = np.empty(N, np.float32)
    charg = np.empty(N, np.float32)
    for c in range(N_CORES):
        asl = slice(c * meta["APC"], (c + 1) * meta["APC"])
        prech[meta["perm"][asl]] = res[c]["out"][0]
        charg[meta["perm"][asl]] = res[c]["out"][1]
    B, A = meta["B"], meta["A"]
    return species, charg.reshape(B, A), prech.reshape(B, A)



# revision 33
# speedup vs baseline: 14.0681x; 1.3928x over previous
"""Trainium2 Bass kernel for nn_LocalMessagePassing (2-pass GNN message passing).

8 NeuronCores, SPMD, data-parallel over molecules (4096 atoms/core):
- species routing via masked PSUM accumulation (4 species matmuls + K=1
  outer-product bias matmuls into the same accumulator)
- celu(z) = max(z,0)+exp(min(z,0))-1, the -1 folded into next-layer bias
- per pass: slice MLP -> bf16 neigh table -> AllGather -> dma_gather of edge
  contributions (dest-sorted, per-32-dest-block padded) -> segment-sum via
  matmul (mergedT += gathered^T @ (onehot*edge_w)) accumulated in PSUM
- final routed linear (M=1 matmuls) + per-molecule charge correction on chip
"""
import sys
sys.path.insert(0, "/opt/trn_rl_repo")
import math
import numpy as np
import ml_dtypes

import concourse.bacc as bacc
import concourse.mybir as mybir
import concourse.tile as tile
from concourse.alu_op_type import AluOpType

BF16 = mybir.dt.bfloat16
FP32 = mybir.dt.float32
I16 = mybir.dt.int16
I8 = mybir.dt.int8
AF = mybir.ActivationFunctionType
F32R = mybir.dt.float32r

N_CORES = 8
S = 4
CUTOFF = 5.2
D_BLK = 32
GSUB = 7            # 128-idx chunks per dma_gather (57 ring entries)


def split_multi_waits(nc):
    """This walrus build allows one sync-wait per instruction; hoist extras
    onto same-engine NOPs placed immediately before."""
    cnt = 0
    for fn in nc.m.functions:
        for bb in fn.blocks:
            out = []
            changed = False
            for ins in bb.instructions:
                si = ins.sync_info
                if si is not None and len(si.on_wait) > 1:
                    waits = list(si.on_wait)
                    for w in waits[:-1]:
                        cnt += 1
                        out.append(mybir.InstNoOp(
                            name=f"wsplit-{cnt}", engine=ins.engine,
                            bass_nofuse=True,
                            sync_info=mybir.SyncInfo(on_wait=[w], on_update=[]),
                        ))
                    ins.sync_info = mybir.SyncInfo(
                        on_wait=[waits[-1]], on_update=list(si.on_update))
                    changed = True
                out.append(ins)
            if changed:
                bb.instructions = out
    return cnt


# ------------------------------------------------------------- blob layout
# All per-core inputs are packed into ONE f32 dram tensor; per-operand
# overhead of the tunneled PJRT execute (~1.6 ms/operand) dominated the
# baseline runtime.
def blob_layout(apc, f_in, h_dim, mo, no, c_blk):
    nblocks = apc // D_BLK
    n_chunks = nblocks * c_blk
    ngi = (n_chunks + GSUB - 1) // GSUB
    padn_g = ngi * GSUB * 128
    secs = [
        ("xT", (f_in, apc), ml_dtypes.bfloat16),
        ("spid", (128, apc), np.int8),
        ("w_p0w1", (S, f_in, h_dim), ml_dtypes.bfloat16),
        ("w_p0w2", (S, 2, 128, mo), ml_dtypes.bfloat16),
        ("w_n0", (S, mo, no), ml_dtypes.bfloat16),
        ("w_p1w1", (S, 2, 128, h_dim), ml_dtypes.bfloat16),
        ("w_p1w2", (S, 2, 128, mo), ml_dtypes.bfloat16),
        ("w_n1", (S, mo, no), ml_dtypes.bfloat16),
        ("w_f", (S, 2, 128, 1), ml_dtypes.bfloat16),
        ("b_p0b1", (S, 1, h_dim), np.float32),
        ("b_p0b2", (S, 1, mo), np.float32),
        ("b_n0", (S, 1, no), np.float32),
        ("b_p1b1", (S, 1, h_dim), np.float32),
        ("b_p1b2", (S, 1, mo), np.float32),
        ("b_n1", (S, 1, no), np.float32),
        ("b_f", (S, 1, 1), np.float32),
        ("eidx", (16, padn_g // 16), np.int16),
        ("destid", (128, n_chunks), ml_dtypes.bfloat16),
        ("wtile", (128, n_chunks), ml_dtypes.bfloat16),
        ("iota32", (128, D_BLK), ml_dtypes.bfloat16),
        ("tc_in", (1, apc // 128), np.float32),
        ("ident", (128, 128), np.float32),
    ]
    layout = {}
    off = 0
    for name, shape, dt in secs:
        n_elem = int(np.prod(shape))
        n_f32 = n_elem * np.dtype(dt).itemsize // 4
        layout[name] = (off, shape, dt)
        off += (n_f32 + 15) // 16 * 16
    off = (off + 127) // 128 * 128
    return layout, off


# ---------------------------------------------------------------- program
def build_program(apc, f_in, h_dim, mo, no, c_blk, *, collect=True,
                  gather=True, edge=True, mlp=True, loads=True, final=True):
    nblocks = apc // D_BLK
    n_chunks = nblocks * c_blk
    ngi = (n_chunks + GSUB - 1) // GSUB
    padn_g = ngi * GSUB * 128
    n_tab = N_CORES * apc
    CT = 512
    ncts = apc // CT
    hck = h_dim // 128

    nc = bacc.Bacc("TRN2", target_bir_lowering=False, debug=False,
                   num_devices=N_CORES)

    layout, tot = blob_layout(apc, f_in, h_dim, mo, no, c_blk)
    # [128, W] shape: a 1-D input tensor makes the runtime's per-exec input
    # handling pathologically slow (~+14 ms); 2-D is handled as a normal
    # parallel transfer.
    blob2d = nc.dram_tensor("blob", [128, tot // 128], FP32,
                            kind="ExternalInput").ap()
    blob = blob2d.rearrange("p w -> (p w)")
    _mydt = {np.float32: FP32, ml_dtypes.bfloat16: BF16, np.int16: I16,
             np.int8: I8}

    def din(name, shape, dt):
        off, lshape, ldt = layout[name]
        assert tuple(shape) == tuple(lshape) and _mydt[ldt] == dt
        n_elem = int(np.prod(shape))
        n_f32 = n_elem * np.dtype(ldt).itemsize // 4
        ap = blob[off:off + n_f32]
        if dt != FP32:
            ap = ap.bitcast(dt)
        dims = " ".join(f"d{i}" for i in range(len(shape)))
        return ap.rearrange(f"({dims}) -> {dims}",
                            **{f"d{i}": s for i, s in enumerate(shape)})

    xT_in = din("xT", [f_in, apc], BF16)
    spid_in = din("spid", [128, apc], I8)
    w_p0w1 = din("w_p0w1", [S, f_in, h_dim], BF16)
    w_p0w2 = din("w_p0w2", [S, 2, 128, mo], BF16)
    w_n0 = din("w_n0", [S, mo, no], BF16)
    w_p1w1 = din("w_p1w1", [S, 2, 128, h_dim], BF16)
    w_p1w2 = din("w_p1w2", [S, 2, 128, mo], BF16)
    w_n1 = din("w_n1", [S, mo, no], BF16)
    w_f = din("w_f", [S, 2, 128, 1], BF16)
    b_p0b1 = din("b_p0b1", [S, 1, h_dim], FP32)
    b_p0b2 = din("b_p0b2", [S, 1, mo], FP32)
    b_n0 = din("b_n0", [S, 1, no], FP32)
    b_p1b1 = din("b_p1b1", [S, 1, h_dim], FP32)
    b_p1b2 = din("b_p1b2", [S, 1, mo], FP32)
    b_n1 = din("b_n1", [S, 1, no], FP32)
    b_f = din("b_f", [S, 1, 1], FP32)
    eidx = din("eidx", [16, padn_g // 16], I16)
    destid = din("destid", [128, n_chunks], BF16)
    wtile_in = din("wtile", [128, n_chunks], BF16)
    iota32 = din("iota32", [128, D_BLK], BF16)
    tc_in = din("tc_in", [1, apc // 128], FP32)
    idin = din("ident", [128, 128], FP32)

    out_t = nc.dram_tensor("out", [2, apc], FP32, kind="ExternalOutput").ap()
    prech_out = out_t[0:1, :]
    charg_out = out_t[1:2, :]

    ntab_loc = [nc.dram_tensor(f"ntl{p}", [apc, no], BF16).ap() for p in range(2)]
    ntab = [nc.dram_tensor(f"ntab{p}", [n_tab, no], BF16, addr_space="Shared").ap()
            for p in range(2)]

    with tile.TileContext(nc) as tc:
        import contextlib
        with contextlib.ExitStack() as ctx:
            persist = ctx.enter_context(tc.tile_pool(name="persist", bufs=1))
            mlp = ctx.enter_context(tc.tile_pool(name="mlp", bufs=2))
            xmp = ctx.enter_context(tc.tile_pool(name="xmp", bufs=1))
            psA = ctx.enter_context(tc.tile_pool(name="psA", bufs=2, space="PSUM"))
            psT = ctx.enter_context(tc.tile_pool(name="psT", bufs=2, space="PSUM"))
            gat = ctx.enter_context(tc.tile_pool(name="gat", bufs=3))
            mkp = ctx.enter_context(tc.tile_pool(name="mkp", bufs=4))

            def loadp(ap_in, shape, dt, name):
                t = persist.tile(shape, dt, tag=name)
                if loads:
                    nc.sync.dma_start(t[:], ap_in)
                else:
                    nc.vector.memset(t[:], 0.125)
                return t

            spid_t = loadp(spid_in[:, :], [128, apc], I8, "spid")
            iota_t = loadp(iota32[:, :], [128, D_BLK], BF16, "iota")
            destid_t = loadp(destid[:, :], [128, n_chunks], BF16, "destid")
            wtile = loadp(wtile_in[:, :], [128, n_chunks], BF16, "wtile")
            tc_t = loadp(tc_in[:, :], [1, apc // 128], FP32, "tc")
            ident_t = loadp(idin[:, :], [128, 128], FP32, "ident")

            # edge-gather indices: stored once as [16, X] in the blob,
            # replicated to 128 partitions on chip (log-doubling SBUF DMAs)
            eidx_t = persist.tile([128, padn_g // 16], I16, tag="eidxf")
            if loads:
                nc.sync.dma_start(eidx_t[0:16, :], eidx)
                for k in (16, 32, 64):
                    nc.sync.dma_start(eidx_t[k:2 * k, :], eidx_t[0:k, :])
            else:
                nc.vector.memset(eidx_t[:], 0)

            # simpler: explicit loads
            def wload3(ap_src, k, s_count, m, name):
                # src AP pre-rearranged to [k, s, m]
                t = persist.tile([k, s_count * m], ap_src.dtype, tag=name)
                if loads:
                    nc.sync.dma_start(
                        t[:].rearrange("k (s m) -> k s m", s=s_count), ap_src)
                else:
                    nc.vector.memset(t[:], 0.125)
                return t

            def wload4(ap_src, k, s_count, c, m, name):
                # src AP pre-rearranged to [k, s, c, m]
                t = persist.tile([k, s_count * c * m], ap_src.dtype, tag=name)
                if loads:
                    nc.sync.dma_start(
                        t[:].rearrange("k (s c m) -> k s c m", s=s_count, c=c),
                        ap_src)
                else:
                    nc.vector.memset(t[:], 0.125)
                return t

            w1t = [wload3(w_p0w1[:, :, :].rearrange("s k m -> k s m"),
                          f_in, S, h_dim, "w1t0"),
                   wload4(w_p1w1[:, :, :, :].rearrange("s c k m -> k s c m"),
                          128, S, 2, h_dim, "w1t1")]
            w2t = [wload4(w_p0w2[:, :, :, :].rearrange("s c k m -> k s c m"),
                          128, S, 2, mo, "w2t0"),
                   wload4(w_p1w2[:, :, :, :].rearrange("s c k m -> k s c m"),
                          128, S, 2, mo, "w2t1")]
            nwt = [wload3(w_n0[:, :, :].rearrange("s k m -> k s m"),
                          mo, S, no, "nwt0"),
                   wload3(w_n1[:, :, :].rearrange("s k m -> k s m"),
                          mo, S, no, "nwt1")]
            fwt = wload4(w_f[:, :, :, :].rearrange("s c k m -> k s c m"),
                         128, S, 2, 1, "fwt")
            b1t = [wload3(b_p0b1[:, :, :].rearrange("s k m -> k s m"), 1, S, h_dim, "b1t0"),
                   wload3(b_p1b1[:, :, :].rearrange("s k m -> k s m"), 1, S, h_dim, "b1t1")]
            b2t = [wload3(b_p0b2[:, :, :].rearrange("s k m -> k s m"), 1, S, mo, "b2t0"),
                   wload3(b_p1b2[:, :, :].rearrange("s k m -> k s m"), 1, S, mo, "b2t1")]
            nbt = [wload3(b_n0[:, :, :].rearrange("s k m -> k s m"), 1, S, no, "nbt0"),
                   wload3(b_n1[:, :, :].rearrange("s k m -> k s m"), 1, S, no, "nbt1")]
            fbt = wload3(b_f[:, :, :].rearrange("s k m -> k s m"), 1, S, 1, "fbt")

            internT = [persist.tile([mo, apc], FP32, tag=f"internT{p}",
                                    name=f"internT{p}")
                       for p in range(2)]
            mergedT = persist.tile([no, apc], FP32, tag="mergedT")
            if not edge:
                nc.vector.memset(mergedT[:], 0.125)
            if not mlp:
                for p in range(2):
                    nc.vector.memset(internT[p][:], 0.125)
                stub = persist.tile([128, no], FP32, tag="stub")
                nc.vector.memset(stub[:], 0.125)
                for p in range(2):
                    for r0 in range(0, apc, 128):
                        nc.sync.dma_start(ntab_loc[p][r0:r0 + 128, :], stub[:])

            mskp = ctx.enter_context(tc.tile_pool(name="mskp", bufs=1))
            _cur_msk = {}

            def build_masks(ct):
                for s in range(S):
                    mt = mskp.tile([128, CT], FP32, tag=f"msk{s}", name=f"msk{s}")
                    nc.vector.tensor_scalar(
                        mt[:], spid_t[:, ct * CT:ct * CT + CT], float(s), None,
                        AluOpType.is_equal)
                    _cur_msk[s] = mt

            def msl(s, ct):
                return _cur_msk[s][:]

            def msl0(s, ct):
                return _cur_msk[s][0:1, :]

            def mlp_phase(p):
                kck = 1 if p == 0 else 2
                for ct in range(ncts):
                    asl = slice(ct * CT, ct * CT + CT)
                    build_masks(ct)
                    if p == 0:
                        xseg = mlp.tile([128, CT], BF16, tag="xseg")
                        nc.sync.dma_start(xseg[:], xT_in[:, asl])
                    # masked inputs for this column tile
                    xms = {}
                    for s in range(S):
                        for kc in range(kck):
                            xm = xmp.tile([128, CT], BF16, tag=f"xm{s}_{kc}")
                            src = (xseg if p == 0
                                   else (internT[0] if kc == 0 else mergedT))
                            nc.vector.tensor_tensor(
                                xm[:], (src[:] if p == 0 else src[:, asl]),
                                msl(s, ct), AluOpType.mult)
                            xms[s, kc] = xm
                    hts = []
                    for hc in range(hck):
                        z1 = psA.tile([128, CT], FP32, tag="zz")
                        for s in range(S):
                            nc.tensor.matmul(
                                z1[:],
                                b1t[p][0:1, s * h_dim + hc * 128:
                                       s * h_dim + hc * 128 + 128],
                                msl0(s, ct), start=(s == 0), stop=False)
                        for s in range(S):
                            for kc in range(kck):
                                if p == 0:
                                    lhsT = w1t[0][:, s * h_dim + hc * 128:
                                                  s * h_dim + hc * 128 + 128]
                                else:
                                    base = s * 2 * h_dim + kc * h_dim + hc * 128
                                    lhsT = w1t[1][:, base:base + 128]
                                nc.tensor.matmul(
                                    z1[:], lhsT,
                                    xms[s, kc][:],
                                    start=False,
                                    stop=(s == S - 1 and kc == kck - 1))
                        mn = mlp.tile([128, CT], FP32, tag="mn")
                        nc.vector.tensor_scalar_min(mn[:], z1[:], 0.0)
                        ex = mlp.tile([128, CT], FP32, tag="ex")
                        nc.scalar.activation(ex[:], mn[:], AF.Exp)
                        ht = mlp.tile([128, CT], FP32, tag="ht")
                        nc.vector.tensor_scalar_max(ht[:], z1[:], 0.0)
                        nc.vector.tensor_tensor(ht[:], ht[:], ex[:], AluOpType.add)
                        hts.append(ht)
                    z2 = psA.tile([128, CT], FP32, tag="zz")
                    for s in range(S):
                        nc.tensor.matmul(
                            z2[:], b2t[p][0:1, s * mo:s * mo + 128],
                            msl0(s, ct), start=(s == 0), stop=False)
                    for s in range(S):
                        for hc in range(hck):
                            hm = mlp.tile([128, CT], BF16, tag="hm")
                            nc.vector.tensor_tensor(
                                hm[:], hts[hc][:], msl(s, ct), AluOpType.mult)
                            base = s * 2 * mo + hc * mo
                            nc.tensor.matmul(
                                z2[:], w2t[p][:, base:base + 128],
                                hm[:],
                                start=False, stop=(s == S - 1 and hc == hck - 1))
                    nc.vector.tensor_copy(internT[p][:, asl], z2[:])
                    zn = psA.tile([128, CT], FP32, tag="zz")
                    for s in range(S):
                        nc.tensor.matmul(
                            zn[:], nbt[p][0:1, s * no:s * no + 128],
                            msl0(s, ct), start=(s == 0), stop=False)
                    for s in range(S):
                        im = mlp.tile([128, CT], BF16, tag="im")
                        nc.vector.tensor_tensor(
                            im[:], internT[p][:, asl], msl(s, ct), AluOpType.mult)
                        nc.tensor.matmul(
                            zn[:], nwt[p][:, s * no:s * no + 128],
                            im[:],
                            start=False, stop=(s == S - 1))
                    nT = mlp.tile([128, CT], FP32, tag="nT")
                    nc.vector.tensor_copy(nT[:], zn[:])
                    for q in range(CT // 128):
                        pt = psT.tile([128, 128], FP32, tag="pt")
                        nc.tensor.transpose(pt[:], nT[:, q * 128:q * 128 + 128],
                                            ident_t[:])
                        rowt = mlp.tile([128, 128], BF16, tag="rowt")
                        nc.vector.tensor_copy(rowt[:], pt[:])
                        r0 = ct * CT + q * 128
                        nc.sync.dma_start(ntab_loc[p][r0:r0 + 128, :], rowt[:])

            def edge_phase(p):
                pscols = 512
                bpp = pscols // D_BLK       # blocks per psum tile (16)
                psm = None
                for gi in range(ngi):
                    gt = gat.tile([128, GSUB, no], BF16, tag="gt")
                    if gather:
                        nc.gpsimd.dma_gather(
                            gt[:], ntab[p][:, :],
                            eidx_t[:, gi * GSUB * 8:(gi + 1) * GSUB * 8],
                            num_idxs=GSUB * 128, num_idxs_reg=GSUB * 128,
                            elem_size=no)
                    else:
                        nc.vector.memset(gt[:], 0.125)
                    c0 = gi * GSUB
                    nsub = min(GSUB, n_chunks - c0)
                    if nsub <= 0:
                        continue
                    csl = slice(c0, c0 + nsub)
                    mk = mkp.tile([128, GSUB * D_BLK], BF16, tag="mk")
                    nc.vector.tensor_tensor(
                        mk[:, :nsub * D_BLK].rearrange("p (c d) -> p c d", d=D_BLK),
                        destid_t[:, csl].broadcast_to([128, nsub, D_BLK]),
                        iota_t[:].rearrange("p (x d) -> p x d", x=1)
                              .broadcast_to([128, nsub, D_BLK]),
                        AluOpType.is_equal)
                    wm = mkp.tile([128, GSUB * D_BLK], BF16, tag="wm")
                    nc.vector.tensor_tensor(
                        wm[:, :nsub * D_BLK].rearrange("p (c d) -> p c d", d=D_BLK),
                        mk[:, :nsub * D_BLK].rearrange("p (c d) -> p c d", d=D_BLK),
                        wtile[:, csl].broadcast_to([128, nsub, D_BLK]),
                        AluOpType.mult)
                    for sub in range(nsub):
                        g = c0 + sub
                        blk = g // c_blk
                        cin = g % c_blk
                        if blk % bpp == 0 and cin == 0:
                            psm = psT.tile([128, pscols], FP32, tag="psm")
                        col0 = (blk % bpp) * D_BLK
                        nc.tensor.matmul(
                            psm[:, col0:col0 + D_BLK],
                            gt[:, sub, :],
                            wm[:, sub * D_BLK:sub * D_BLK + D_BLK],
                            start=(cin == 0), stop=(cin == c_blk - 1))
                        if cin == c_blk - 1 and blk % bpp == bpp - 1:
                            grp = blk // bpp
                            nc.vector.tensor_copy(
                                mergedT[:, grp * pscols:(grp + 1) * pscols],
                                psm[:])

            def final_phase():
                prech = persist.tile([1, apc], FP32, tag="prech")
                if not final:
                    nc.vector.memset(prech[:], 0.125)
                    nc.sync.dma_start(prech_out[:, :], prech[:])
                    nc.sync.dma_start(charg_out[:, :], prech[:])
                    return
                for ct in range(ncts):
                    build_masks(ct)
                    zf = psT.tile([1, CT], FP32, tag="psm")
                    for s in range(S):
                        nc.tensor.matmul(
                            zf[:], fbt[0:1, s:s + 1], msl0(s, ct),
                            start=(s == 0), stop=False)
                    for s in range(S):
                        for kc in range(2):
                            src = internT[1] if kc == 0 else mergedT
                            xm = mlp.tile([128, CT], BF16, tag="xmf")
                            nc.vector.tensor_tensor(
                                xm[:], src[:, ct * CT:ct * CT + CT],
                                msl(s, ct), AluOpType.mult)
                            nc.tensor.matmul(
                                zf[:], fwt[:, s * 2 + kc:s * 2 + kc + 1],
                                xm[:],
                                start=False,
                                stop=(s == S - 1 and kc == 1))
                    nc.vector.tensor_copy(prech[0:1, ct * CT:ct * CT + CT], zf[:])
                nc.sync.dma_start(prech_out[:, :], prech[:])
                nmol = apc // 128
                tpre = mlp.tile([1, nmol], FP32, tag="tpre")
                nc.vector.tensor_reduce(
                    tpre[0:1, :],
                    prech[0:1, :].rearrange("p (m a) -> p m a", a=128),
                    mybir.AxisListType.X, AluOpType.add)
                corr = mlp.tile([1, nmol], FP32, tag="corr")
                nc.vector.tensor_tensor(corr[:], tc_t[:], tpre[:],
                                        AluOpType.subtract)
                nc.vector.tensor_scalar_mul(corr[:], corr[:], 1.0 / 128.0)
                nc.vector.tensor_tensor(
                    prech[0:1, :].rearrange("p (m a) -> p m a", a=128),
                    prech[0:1, :].rearrange("p (m a) -> p m a", a=128),
                    corr[0:1, :].broadcast_to([1, nmol, 128]),
                    AluOpType.add)
                nc.sync.dma_start(charg_out[:, :], prech[:])

            for p in range(2):
                if mlp:
                    mlp_phase(p)
                if collect:
                    nc.gpsimd.collective_compute(
                        "AllGather", AluOpType.bypass,
                        replica_groups=[list(range(N_CORES))],
                        ins=[ntab_loc[p]], outs=[ntab[p]])
                if edge:
                    edge_phase(p)
            final_phase()

    nc.compile()
    split_multi_waits(nc)
    return nc


# ---------------------------------------------------------------- host prep
def _wrap_idx(flat_idx):
    n = len(flat_idx)
    a = np.zeros((16, (n + 15) // 16), np.int16)
    a[np.arange(n) % 16, np.arange(n) // 16] = flat_idx
    return a


def prepare_inputs(species, in_features, atom_index12, distances, total_charges,
                   p0_w1, p0_b1, p0_w2, p0_b2, n0_w, n0_b,
                   p1_w1, p1_b1, p1_w2, p1_b2, n1_w, n1_b,
                   f_w, f_b, prefactor, factor):
    B, A = np.asarray(species).shape
    N = B * A
    F_IN = np.asarray(in_features).shape[-1]
    H = np.asarray(p0_w1).shape[-1]
    MO = np.asarray(p0_w2).shape[-1]
    NO = np.asarray(n0_w).shape[-1]
    APC = N // N_CORES
    sp = np.asarray(species).reshape(-1).astype(np.int64)
    feats = np.asarray(in_features, np.float32).reshape(N, F_IN)

    # atoms keep their natural order (core c owns [c*APC, (c+1)*APC));
    # species routing is fully mask-based so no sorting is needed, and
    # molecule boundaries (128-atom groups) stay intact for the charge
    # correction.
    perm = np.arange(N)
    inv = perm
    sp_sorted = sp

    i0 = inv[np.asarray(atom_index12[0], np.int64)]
    i1 = inv[np.asarray(atom_index12[1], np.int64)]
    dd = np.asarray(distances, np.float32)
    dest = np.concatenate([i0, i1])
    src = np.concatenate([i1, i0])
    ddist = np.concatenate([dd, dd])

    nblocks = APC // D_BLK
    dcore = dest // APC
    dloc = dest - dcore * APC
    dblk = dloc // D_BLK

    counts = np.bincount(dcore * nblocks + dblk, minlength=N_CORES * nblocks)
    c_blk = int(np.ceil(counts.max() / 128.0))
    n_chunks = nblocks * c_blk
    ngi = (n_chunks + GSUB - 1) // GSUB
    padn_g = ngi * GSUB * 128
    slots = n_chunks * 128

    key = dcore * nblocks + dblk
    order = np.argsort(key, kind="stable")
    bounds = np.searchsorted(key[order], np.arange(N_CORES * nblocks + 1))

    # host-precomputed edge weight (decay * cosine cutoff), fp64
    pf = float(np.asarray(prefactor)); fc = float(np.asarray(factor))
    dd64 = ddist.astype(np.float64)
    ew = (pf * pf) * np.exp(-(fc * fc) * dd64) * np.where(
        dd64 < CUTOFF, 0.5 * np.cos(np.pi * dd64 / CUTOFF) + 0.5, 0.0)

    eidx_np = np.zeros((N_CORES, 16, padn_g // 16), np.int16)
    destid_np = np.zeros((N_CORES, 128, n_chunks), ml_dtypes.bfloat16)
    wtile_np = np.zeros((N_CORES, 128, n_chunks), ml_dtypes.bfloat16)
    j = np.arange(slots)
    for c in range(N_CORES):
        idx_flat = np.zeros(slots, np.int64)
        did_flat = np.full(slots, float(D_BLK), np.float32)   # pad -> no match
        w_flat = np.zeros(slots, np.float64)
        for b in range(nblocks):
            g0, g1 = bounds[c * nblocks + b], bounds[c * nblocks + b + 1]
            cnt = g1 - g0
            s0 = b * c_blk * 128
            sel = order[g0:g1]
            sel = sel[np.argsort(src[sel], kind="stable")]  # gather locality
            idx_flat[s0:s0 + cnt] = src[sel]
            did_flat[s0:s0 + cnt] = (dloc[sel] % D_BLK).astype(np.float32)
            w_flat[s0:s0 + cnt] = ew[sel]
        eidx_np[c] = _wrap_idx(np.concatenate(
            [idx_flat, np.zeros(padn_g - slots, np.int64)]).astype(np.int16))
        destid_np[c, j % 128, j // 128] = did_flat.astype(ml_dtypes.bfloat16)
        wtile_np[c, j % 128, j // 128] = w_flat.astype(ml_dtypes.bfloat16)

    def f32(x):
        return np.ascontiguousarray(np.asarray(x, np.float32))

    p0b2_adj = np.asarray(p0_b2, np.float64) - np.asarray(p0_w2, np.float64).sum(1)
    p1b2_adj = np.asarray(p1_b2, np.float64) - np.asarray(p1_w2, np.float64).sum(1)

    def kchunk(w):  # [S, 2k, m] -> [S, 2, 128, m]
        w = np.asarray(w, np.float32)
        return w.reshape(w.shape[0], 2, 128, w.shape[-1])

    common = {
        "w_p0w1": f32(p0_w1), "w_p0w2": f32(kchunk(p0_w2)), "w_n0": f32(n0_w),
        "w_p1w1": f32(kchunk(p1_w1)), "w_p1w2": f32(kchunk(p1_w2)),
        "w_n1": f32(n1_w), "w_f": f32(kchunk(f_w)),
        "b_p0b1": f32(np.asarray(p0_b1))[:, None, :],
        "b_p0b2": f32(p0b2_adj)[:, None, :],
        "b_n0": f32(np.asarray(n0_b))[:, None, :],
        "b_p1b1": f32(np.asarray(p1_b1))[:, None, :],
        "b_p1b2": f32(p1b2_adj)[:, None, :],
        "b_n1": f32(np.asarray(n1_b))[:, None, :],
        "b_f": f32(np.asarray(f_b))[:, None, :],
        "iota32": np.tile(np.arange(D_BLK, dtype=np.float32).astype(
            ml_dtypes.bfloat16)[None, :], (128, 1)),
        "ident": np.eye(128, dtype=np.float32),
    }

    tc_np = np.asarray(total_charges, np.float32)
    layout, tot = blob_layout(APC, F_IN, H, MO, NO, c_blk)

    def pack(vals):
        blob = np.zeros(tot, np.float32)
        for name, (off, shape, dt) in layout.items():
            a = np.ascontiguousarray(np.asarray(vals[name], dt)).reshape(shape)
            raw = a.ravel().view(np.float32)
            blob[off:off + raw.size] = raw
        return blob.reshape(128, tot // 128)

    in_maps = []
    for c in range(N_CORES):
        asl = slice(c * APC, (c + 1) * APC)
        spc = sp_sorted[asl]
        xT = np.ascontiguousarray(feats[perm[asl]].T)
        spid_c = np.tile(spc.astype(np.float32)[None, :], (128, 1)).astype(
            ml_dtypes.bfloat16)
        vals = {
            "xT": xT, "spid": spid_c,
            "eidx": eidx_np[c], "destid": destid_np[c], "wtile": wtile_np[c],
            "tc_in": tc_np[c * (B // N_CORES):(c + 1) * (B // N_CORES)][None, :],
            **common,
        }
        in_maps.append({"blob": pack(vals)})
    meta = dict(perm=perm, B=B, A=A, APC=APC, c_blk=c_blk,
                F_IN=F_IN, H=H, MO=MO, NO=NO)
    return in_maps, meta


# ---------------------------------------------------------------- runner
class SpmdRunner:
    def __init__(self, nc, n_cores=N_CORES):
        import jax
        from concourse import bass2jax
        from concourse.bass2jax import _bass_exec_p, install_neuronx_cc_hook
        from jax.sharding import Mesh, PartitionSpec
        from jax.experimental.shard_map import shard_map
        install_neuronx_cc_hook()
        self.jax = jax
        self.nc = nc
        self.n_cores = n_cores
        in_names, out_names, out_avals, zero_outs = [], [], [], []
        partition_name = (nc.partition_id_tensor.name
                          if nc.partition_id_tensor else None)
        for alloc in nc.m.functions[0].allocations:
            if not isinstance(alloc, mybir.MemoryLocationSet):
                continue
            name = alloc.memorylocations[0].name
            if alloc.kind == "ExternalInput":
                if name != partition_name:
                    in_names.append(name)
            elif alloc.kind == "ExternalOutput":
                shape = tuple(alloc.tensor_shape)
                dtype = mybir.dt.np(alloc.dtype)
                out_names.append(name)
                out_avals.append(jax.core.ShapedArray(shape, dtype))
                zero_outs.append(np.zeros(shape, dtype))
        n_params = len(in_names)
        all_in = in_names + out_names
        if partition_name is not None:
            all_in.append(partition_name)

        def _body(*args):
            operands = list(args)
            if partition_name is not None:
                operands.append(bass2jax.partition_id_tensor())
            outs = _bass_exec_p.bind(
                *operands, out_avals=tuple(out_avals), in_names=tuple(all_in),
                out_names=tuple(out_names), lowering_input_output_aliases=(),
                sim_require_finite=True, sim_require_nnan=True, nc=nc)
            return tuple(outs)

        devices = jax.devices()[:n_cores]
        mesh = Mesh(np.asarray(devices), ("core",))
        in_specs = (PartitionSpec("core"),) * (n_params + len(out_names))
        out_specs = (PartitionSpec("core"),) * len(out_names)
        self._fn = jax.jit(
            shard_map(_body, mesh=mesh, in_specs=in_specs,
                      out_specs=out_specs, check_rep=False),
            keep_unused=True)
        self.in_names, self.out_names = in_names, out_names
        self.out_avals, self.zero_outs = out_avals, zero_outs
        self.n_params = n_params

    def prepare(self, in_maps):
        per_core = [[np.asarray(m[n]) for n in self.in_names] for m in in_maps]
        concat_in = [
            np.concatenate([per_core[c][i] for c in range(self.n_cores)], axis=0)
            for i in range(self.n_params)]
        concat_zeros = [
            np.zeros((self.n_cores * z.shape[0], *z.shape[1:]), z.dtype)
            for z in self.zero_outs]
        args = [self.jax.device_put(a) for a in concat_in + concat_zeros]
        for a in args:
            a.block_until_ready()
        self._args = args

    def run(self):
        outs = self._fn(*self._args)
        self.jax.block_until_ready(outs)
        return outs

    def run_async(self):
        return self._fn(*self._args)

    def results(self, outs):
        return [
            {name: np.asarray(outs[i]).reshape(
                self.n_cores, *self.out_avals[i].shape)[c]
             for i, name in enumerate(self.out_names)}
            for c in range(self.n_cores)]


def jax_block(x):
    import jax
    jax.block_until_ready(x)


_CACHE = {}


def _get_runner(apc, f_in, h_dim, mo, no, c_blk):
    key = (apc, f_in, h_dim, mo, no, c_blk)
    if key not in _CACHE:
        nc = build_program(apc, f_in, h_dim, mo, no, c_blk)
        _CACHE[key] = SpmdRunner(nc, N_CORES)
    return _CACHE[key]


def kernel(**inputs):
    species = inputs["species"]
    in_maps, meta = prepare_inputs(**inputs)
    r = _get_runner(meta["APC"], meta["F_IN"], meta["H"], meta["MO"],
                    meta["NO"], meta["c_blk"])
    r.prepare(in_maps)
    outs = r.run()
    res = r.results(outs)
    N = meta["B"] * meta["A"]
    prech = np.empty(N, np.float32)
    charg = np.empty(N, np.float32)
    for c in range(N_CORES):
        asl = slice(c * meta["APC"], (c + 1) * meta["APC"])
        prech[meta["perm"][asl]] = res[c]["out"][0]
        charg[meta["perm"][asl]] = res[c]["out"][1]
    B, A = meta["B"], meta["A"]
    return species, charg.reshape(B, A), prech.reshape(B, A)

